# revision 1
# baseline (speedup 1.0000x reference)
"""Trainium2 Bass kernel for nn_EnhancedGNN (3-block GCN + BN + MLP head).

Strategy (8 NeuronCores, node-partitioned graph parallel):
  - Nodes are partitioned contiguously across the 8 cores (12500 each,
    padded to 12544 = 98*128). Each core owns all edges whose dst lies in
    its partition. Self-loops never enter the message stream: their
    contribution (dinv^2 * h@W) is added per window from the local shard.
  - Per GCN layer, each core holds a full fp16 feature table
    H~ = dinv * (h @ W) in its DRAM, replicated NOT via collectives but by
    an XOR-pair RDMA exchange: step m swaps half-shards with core (own^m)
    through SBUF slots (remote_dma_broadcast), paced by per-round ack
    semaphores; cross-die dests carry a ^2 routing compensation. Each
    core's table stores shard s at region (s ^ own), row-interleaved so
    drains write contiguous 25KB runs. Messages are fetched with
    dma_gather (random 256B rows, int16 group-relative indices, trailing
    pads negative + per-core valid counts in a register stream).
  - Segment-sum by dst is a one-hot matmul: for each chunk of 128
    dst-sorted messages, S[msg, dstslot] = (dstrel[msg] == iota[slot])
    built on DVE, then PSUM += S^T @ msgs on TensorE. dinv[dst] is applied
    on PSUM evacuation; dinv[src] is folded into the table rows; the GCN
    bias is absorbed by the following BatchNorm.
  - h lives FEATURE-major ([feat, node]) so the h @ W table matmuls need
    no transpose, and BN+ReLU fuses into one per-partition scale/bias op
    (alternating between ACT and DVE to balance engine load). BN stats
    are column vectors from ones-matmuls, summed across cores by tiny
    RDMA broadcasts with per-phase semaphores.
"""

import numpy as np

import concourse.bacc as bacc
import concourse.mybir as mybir
import concourse.tile as tile
from concourse.bass import AP
from concourse.bass_utils import run_bass_kernel_spmd
from concourse.masks import make_identity

F32 = mybir.dt.float32
F16 = mybir.dt.float16
I16 = mybir.dt.int16
I32 = mybir.dt.int32

N_NODES = 100000
HID = 128
NCORES = 8
EPS = 1e-5
RG = [list(range(NCORES))]
WINB = 4                         # windows per PSUM batch ([128, 512] tile)
KMAX = 32                        # chunks per dma_gather call


def _set_config(n_nodes=100000, group=32768):
    """Derive all sharding constants (module globals) from the node count."""
    global N_NODES, NSH, TPC, NPAD, TROWS, GROUP, GBASES, GSIZES, NG
    N_NODES = n_nodes
    NSH = N_NODES // NCORES
    TPC = (NSH + 127) // 128
    NPAD = TPC * 128
    TROWS = NCORES * NPAD
    GROUP = group
    GBASES = list(range(0, TROWS, GROUP))
    GSIZES = [min(GROUP, TROWS - b) for b in GBASES]
    NG = len(GBASES)


_set_config()


# ----------------------------------------------------------------------------
# Host-side preprocessing: edge bucketing + per-core streams
# ----------------------------------------------------------------------------

def _preprocess(edge_index, x):
    # Self-loops are excluded from the gather stream: their source rows are
    # the core's own tshard_sb (already in SBUF) and are added on-device as
    # dinv * tshard per window. Degrees still count the loops.
    src = edge_index[0].astype(np.int64)
    dst = edge_index[1].astype(np.int64)

    deg = (np.bincount(dst, minlength=N_NODES) + 1).astype(np.float32)
    dinv = (1.0 / np.sqrt(deg)).astype(np.float32)

    owner = src // NSH
    loc = src % NSH
    # XOR-permuted region + row-interleaved layout: on core c, the shard of
    # core s lives at table rows [(s ^ c) * NPAD, ...), row = (loc%128)*TPC
    # + loc//128 so the RDMA drain writes contiguous 25KB runs per partition.
    rowin = (loc % 128) * TPC + (loc // 128)
    core = dst // NSH
    dloc = dst % NSH
    win = dloc // 128
    drel = dloc % 128

    counts = np.zeros((NCORES, TPC, NG), np.int64)
    percore = []
    for c in range(NCORES):
        m = core == c
        s_c = (owner[m] ^ c) * NPAD + rowin[m]   # per-core table row id
        g_c = s_c // GROUP
        w_c, d_c = win[m], drel[m]
        o = np.lexsort((d_c, g_c, w_c))
        s_c, w_c, g_c, d_c = s_c[o], w_c[o], g_c[o], d_c[o]
        cnt = np.zeros((TPC, NG), np.int64)
        np.add.at(cnt, (w_c, g_c), 1)
        counts[c] = cnt
        percore.append((s_c, d_c, cnt))
    cmax = counts.max(0)
    chunks = -(-cmax // 128)                     # [TPC, NG] ceil
    assert (chunks.sum(1) > 0).all()

    # ---- static call plan shared by every core
    win_total = chunks.sum(1)
    batches = []
    bucket_gids = {}                             # (w, g) -> [chunk gids]
    gid = 0
    ncalls = 0
    nb = -(-TPC // WINB)
    for b in range(nb):
        wins = list(range(b * WINB, min((b + 1) * WINB, TPC)))
        emitted = {w: 0 for w in wins}
        calls = []
        chunk0 = gid
        for g in range(NG):
            pend = []
            for w in wins:
                if chunks[w, g]:
                    bucket_gids.setdefault((w, g), [])
                    pend.extend([w] * chunks[w, g])
            for i0 in range(0, len(pend), KMAX):
                sub = pend[i0:i0 + KMAX]
                ch = []
                for j, w in enumerate(sub):
                    st = emitted[w] == 0
                    emitted[w] += 1
                    # stop is carried by the self-loop identity matmul that
                    # closes each window's accumulation in the evac section
                    ch.append((w, st, False))
                    bucket_gids[(w, g)].append(gid + j)
                calls.append(dict(g=g, k=len(sub), chunks=ch, gid0=gid,
                                  cid=ncalls))
                ncalls += 1
                gid += len(sub)
        batches.append(dict(wins=wins, calls=calls, chunk0=chunk0,
                            nchunks=gid - chunk0))
    tot_chunks = gid

    # ---- per-core streams
    streams = []
    for c in range(NCORES):
        s_c, d_c, cnt = percore[c]
        # offsets of each (w, g) bucket in the lexsorted arrays
        flat = cnt.reshape(-1)
        offs = np.concatenate([[0], np.cumsum(flat)[:-1]]).reshape(TPC, NG)
        # pad slots carry idx -1: the gather ucode skips negative indices,
        # saving the padding fetch bandwidth (S is zero there regardless)
        idx_stream = np.full((tot_chunks, 128), -1, np.int16)
        drel_stream = np.full((tot_chunks, 128), -1.0, np.float32)
        for (w, g), gids in bucket_gids.items():
            n = cnt[w, g]
            ncap = len(gids) * 128
            o0 = offs[w, g]
            vals = np.full(ncap, -1, np.int64)
            vals[:n] = s_c[o0:o0 + n] - GBASES[g]
            dr = np.full(ncap, -1.0, np.float32)
            dr[:n] = d_c[o0:o0 + n]
            idx_stream[gids] = vals.reshape(-1, 128).astype(np.int16)
            drel_stream[gids] = dr.reshape(-1, 128)
        # Negative idxs are skipped by the gather ucode, but only as a
        # trailing suffix: keep -1 for each call's tail pads, remap interior
        # pads to dummy row 0, and pass count = last valid + 1 per call.
        ncnt = np.zeros((1, ncalls), np.int32)
        for b in batches:
            for call in b["calls"]:
                g0, k = call["gid0"], call["k"]
                blk = idx_stream[g0:g0 + k].reshape(-1)
                nz = np.nonzero(blk >= 0)[0]
                L = int(nz[-1]) + 1 if len(nz) else 0
                head = blk[:L]
                head[head < 0] = 0
                blk[:L] = head
                idx_stream[g0:g0 + k] = blk.reshape(k, 128)
                ncnt[0, call["cid"]] = L
        # wrap indices: chunk j, msg p -> [p % 16, j*8 + p//16], replicate x8
        idx16 = idx_stream.reshape(tot_chunks * 8, 16).T          # [16, 8*T]
        idx_wrapped = np.tile(idx16, (8, 1)).copy()               # [128, 8*T]
        drel_cols = np.ascontiguousarray(drel_stream.T).astype(np.float16)
        dv = np.zeros(NPAD, np.float32)
        dv[:NSH] = dinv[c * NSH:(c + 1) * NSH]
        dinv_cols = np.ascontiguousarray(dv.reshape(TPC, 128).T)  # [128, TPC]
        mk = np.zeros(NPAD, np.float32)
        mk[:NSH] = 1.0
        msk_cols = np.ascontiguousarray(mk.reshape(TPC, 128).T)
        xp = np.zeros((NPAD, 2), np.float32)
        xp[:NSH] = x[c * NSH:(c + 1) * NSH]
        xT = np.ascontiguousarray(xp.T).astype(np.float16)        # [2, NPAD]
        streams.append(dict(idxs=idx_wrapped, drel=drel_cols,
                            dinv=dinv_cols, msk=msk_cols, xT=xT, ncnt=ncnt))

    plan = dict(batches=batches, tot_chunks=tot_chunks,
                tot_cols=tot_chunks * 8, ncalls=ncalls,
                maxnch=max(b["nchunks"] for b in batches))
    return plan, streams


# ----------------------------------------------------------------------------
# Device program
# ----------------------------------------------------------------------------

def _build_program(plan):
    nc = bacc.Bacc("TRN2", target_bir_lowering=False, debug=False,
                   enable_asserts=True, num_devices=NCORES)

    def din(name, shape, dt=F32):
        return nc.dram_tensor(name, list(shape), dt, kind="ExternalInput").ap()

    t_idx = din("idxs", [128, plan["tot_cols"]], I16)
    t_ncnt = din("ncnt", [1, plan["ncalls"]], I32)
    t_drel = din("drel", [128, plan["tot_chunks"]], F16)
    t_dinv = din("dinv", [128, TPC])
    t_msk = din("msk", [128, TPC])
    t_xT = din("xT", [2, NPAD], F16)
    t_We = din("We", [2, HID], F16)
    t_W = {1: din("W1", [HID, HID], F16), 2: din("W2", [HID, HID], F16),
           3: din("W3", [HID, HID], F16)}
    t_Wf1 = din("Wf1", [HID, 32], F16)
    t_Wf2 = din("Wf2e", [33, 2], F16)
    t_be = din("be_col", [HID, 1])
    t_g = {i: din(f"g{i}", [HID, 1]) for i in (1, 2, 3)}
    t_bt = {i: din(f"bt{i}", [HID, 1]) for i in (1, 2, 3)}
    t_gf = din("gf", [32, 1])
    t_btf = din("btf", [32, 1])
    t_out = nc.dram_tensor("out", [128, TPC * 2], F32,
                           kind="ExternalOutput").ap()

    from contextlib import ExitStack
    with tile.TileContext(nc) as tc, ExitStack() as st:
        cst = st.enter_context(tc.tile_pool(name="cst", bufs=1))
        sb = st.enter_context(tc.tile_pool(name="sb", bufs=2))
        msgp = st.enter_context(tc.tile_pool(name="msgp", bufs=4))
        ps_agg = st.enter_context(tc.tile_pool(name="ps_agg", bufs=1, space="PSUM"))
        ps_st = st.enter_context(tc.tile_pool(name="ps_st", bufs=1, space="PSUM"))
        ps_a = st.enter_context(tc.tile_pool(name="ps_a", bufs=1, space="PSUM"))
        ps_b = st.enter_context(tc.tile_pool(name="ps_b", bufs=1, space="PSUM"))
        dr = st.enter_context(tc.tile_pool(name="dr", bufs=1, space="DRAM"))
        _emit(nc, tc, plan, locals())
    nc.compile()
    return nc


def _emit(nc, tc, plan, pools):
    cst, sb, msgp = pools["cst"], pools["sb"], pools["msgp"]
    ps_agg, ps_st = pools["ps_agg"], pools["ps_st"]
    ps_a, ps_b, dr = pools["ps_a"], pools["ps_b"], pools["dr"]
    t_idx, t_drel = pools["t_idx"], pools["t_drel"]
    t_ncnt = pools["t_ncnt"]
    t_dinv, t_xT, t_We = pools["t_dinv"], pools["t_xT"], pools["t_We"]
    t_W, t_Wf1, t_Wf2 = pools["t_W"], pools["t_Wf1"], pools["t_Wf2"]
    t_be, t_g, t_bt = pools["t_be"], pools["t_g"], pools["t_bt"]
    t_gf, t_btf = pools["t_gf"], pools["t_btf"]
    t_out = pools["t_out"]
    AO, AF = mybir.AluOpType, mybir.ActivationFunctionType

    # ---- constants
    iota_i = cst.tile([128, 128], I32)
    nc.gpsimd.iota(iota_i[:], pattern=[[1, 128]], base=0, channel_multiplier=0)
    iota_f32 = cst.tile([128, 128], F32)
    nc.vector.tensor_copy(iota_f32[:], iota_i[:])
    iota_f = cst.tile([128, 128], F16)
    nc.vector.tensor_copy(iota_f[:], iota_f32[:])
    ident = cst.tile([128, 128], F16)
    make_identity(nc, ident[:])
    ones_col = cst.tile([128, 1], F16)
    nc.vector.memset(ones_col[:], 1.0)
    eps_col = cst.tile([128, 1], F32)
    nc.vector.memset(eps_col[:], EPS)

    def load_const(t, shape, dt=F32):
        tl = cst.tile(shape, dt, name=f"c_{t.tensor.name}")
        nc.sync.dma_start(tl[:], t[:])
        return tl

    dinv_sb = load_const(t_dinv, [128, TPC])
    ncnt_sb = load_const(t_ncnt, [1, plan["ncalls"]], I32)
    ncnt_reg = nc.gpsimd.alloc_register("ncnt_reg")
    msk_sb = load_const(pools["t_msk"], [128, TPC])
    We_sb = load_const(t_We, [2, HID], F16)
    W_sb = {i: load_const(t_W[i], [HID, HID], F16) for i in (1, 2, 3)}
    Wf1_sb = load_const(t_Wf1, [HID, 32], F16)
    Wf2_sb = load_const(t_Wf2, [33, 2], F16)
    be_sb = load_const(t_be, [HID, 1])
    g_sb = {i: load_const(t_g[i], [HID, 1]) for i in (1, 2, 3)}
    bt_sb = {i: load_const(t_bt[i], [HID, 1]) for i in (1, 2, 3)}
    gf_sb = load_const(t_gf, [32, 1])
    btf_sb = load_const(t_btf, [32, 1])

    h_fm = cst.tile([128, NPAD], F16)        # FEATURE-major h [feat, node]
    agg_big = cst.tile([128, NPAD], F16)     # node-major aggregation output
    fpre_big = cst.tile([128, TPC * 32], F16)
    tshard_sb = cst.tile([128, NPAD], F16)   # staging for the table shard
    xt_sb = cst.tile([2, NPAD], F16)         # embedded input, loaded once
    out_nm = cst.tile([128, TPC * 2], F32)   # node-major staged output
    HTPC = TPC // 2
    slot_t = [cst.tile([128, HTPC * 128], F16, name=f"slot{p}")
              for p in (0, 1)]
    st_l = {i: cst.tile([128, 2], F32, name=f"st{i}") for i in (1, 2, 3, 4)}
    slot_st = {i: cst.tile([128, 14], F32, name=f"sst{i}")
               for i in (1, 2, 3, 4)}

    tables = {i: dr.tile([TROWS, HID], F16, name=f"table{i}")
              for i in (1, 2, 3)}

    import os
    SIM_STUB = bool(os.environ.get("KERNEL_SIM_STUB"))

    # XOR-pair exchange: step m swaps shards with core (own ^ m). Cross-die
    # dests (bit 2) get an extra ^2 from the ucode lane balance; compensate
    # (the interpreter models ideal routing — KERNEL_RDMA_NOCOMP disables it).
    NOCOMP = bool(os.environ.get("KERNEL_RDMA_NOCOMP"))

    def rd(m):
        if NOCOMP:
            return (0, m)
        return (0, m ^ 2 if m >= 4 else m)

    RD1 = {m: [None] * 8 for m in range(1, 8)}
    for m in range(1, 8):
        RD1[m][m] = rd(m)
    RD_ALL = [None] + [rd(k) for k in range(1, 8)]

    nc.vector.memset(st_l[4][:], 0.0)
    for _ in range(4):
        # initialize all three rotating gather buffers: rows of slots the
        # gather skips (trailing pads) are read by the matmul (times S=0)
        mz = msgp.tile([128, KMAX * 128], F16, tag="msg", name="msg")
        nc.vector.memset(mz[:], 0.0)

    rsem_t = [nc.alloc_semaphore("rsem_t0"), nc.alloc_semaphore("rsem_t1")]
    rsem_s = {i: nc.alloc_semaphore(f"rsem_s{i}") for i in (1, 2, 3, 4)}
    lsem_t = nc.alloc_semaphore("lsem_t")
    lsem_s = nc.alloc_semaphore("lsem_s")
    ack_sems = [nc.alloc_semaphore(f"ack{j}") for j in range(14)]
    ack_lsem = nc.alloc_semaphore("ack_lsem")
    psem = nc.alloc_semaphore("psem")
    tsem = nc.alloc_semaphore("tsem")
    cnt = dict(prep=0, send_t=0, arr0=0, arr1=0, drain=0, flush=0)

    def prep_trig(inst):
        inst.then_inc(psem, 1)
        cnt["prep"] += 1
        nc.gpsimd.wait_ge(psem, cnt["prep"])
        nc.gpsimd.trigger_dma(count=1)

    def ts(t):
        return slice(t * 128, (t + 1) * 128)

    def region(table, m):
        return table[m * NPAD:(m + 1) * NPAD, :].rearrange(
            "(p w) f -> p w f", w=TPC)

    def flush_tshard(table):
        """XOR-pair RDMA exchange of tshard_sb into the local table copy.

        Half-shard rounds: round r = (m-1)*2 + h sends window-half h of the
        shard to peer (own ^ m), 2-slot pipelined with a global drain-ack
        barrier gating slot reuse (round r waits acks of rounds <= r-2)."""
        if SIM_STUB:
            # Cost-sim stand-in: the local drains only (the broadcasts and
            # handshake can't be modeled by a single-core timeline sim).
            for m in range(8):
                nc.sync.dma_start(
                    region(table, m),
                    tshard_sb[:].rearrange("p (w f) -> p w f", f=HID))
            return
        F = cnt["flush"]
        cnt["flush"] += 1
        with tc.tile_critical():
            nc.sync.dma_start(
                region(table, 0),
                tshard_sb[:].rearrange("p (w f) -> p w f", f=HID)
            ).then_inc(tsem, 16)
            cnt["drain"] += 16
            for m in range(1, 8):
                for h in (0, 1):
                    j = (m - 1) * 2 + h          # round within this flush
                    # slot-reuse barrier: round j-2 (same flush; or j+12 of
                    # the previous flush) drained on every core
                    if not SIM_STUB and (F > 0 or j >= 2):
                        if j >= 2:
                            nc.gpsimd.wait_ge(ack_sems[j - 2], 14 * (F + 1))
                        else:
                            nc.gpsimd.wait_ge(ack_sems[j + 12], 14 * F)
                    cs = slice(h * HTPC * 128, (h + 1) * HTPC * 128)
                    prep_trig(nc.gpsimd.remote_dma_broadcast(
                        out_ap=slot_t[j % 2][:], in_ap=tshard_sb[:, cs],
                        remote_sem=rsem_t[j % 2], local_sem=lsem_t,
                        rdests=RD1[m]))
                    cnt["send_t"] += 16
                    if j > 0:
                        # ack round j-1 (its drain was issued last iteration)
                        nc.gpsimd.wait_ge(tsem, cnt["drain"])
                        prep_trig(nc.gpsimd.remote_sem_update_broadcast(
                            ack_sems[j - 1], ack_lsem, rdests=RD_ALL))
                    par = j % 2
                    key = "arr0" if par == 0 else "arr1"
                    cnt[key] += 2
                    if not SIM_STUB:
                        nc.sync.wait_ge(rsem_t[par], cnt[key])
                    nc.sync.dma_start(
                        region(table, m)[:, h * HTPC:(h + 1) * HTPC, :],
                        slot_t[par][:].rearrange("p (w f) -> p w f", f=HID)
                    ).then_inc(tsem, 16)
                    cnt["drain"] += 16
            nc.gpsimd.wait_ge(tsem, cnt["drain"])
            prep_trig(nc.gpsimd.remote_sem_update_broadcast(
                ack_sems[13], ack_lsem, rdests=RD_ALL))
            nc.sync.wait_ge(tsem, cnt["drain"])
            nc.gpsimd.wait_ge(lsem_t, cnt["send_t"])

    def exchange_stats(phase):
        """Sum [128, 2] column stats across cores via RDMA broadcasts."""
        gl = sb.tile([128, 2], F32, tag="stv", name=f"glv{phase}")
        if SIM_STUB:
            nc.vector.tensor_copy(gl[:], st_l[phase][:])
            return gl
        with tc.tile_critical():
            for m in range(1, 8):
                prep_trig(nc.gpsimd.remote_dma_broadcast(
                    out_ap=slot_st[phase][:, 2 * (m - 1):2 * m],
                    in_ap=st_l[phase][:], remote_sem=rsem_s[phase],
                    local_sem=lsem_s, rdests=RD1[m]))
            if not SIM_STUB:
                nc.vector.wait_ge(rsem_s[phase], 14)
        nc.vector.tensor_tensor(out=gl[:], in0=st_l[phase][:],
                                in1=slot_st[phase][:, 0:2], op=AO.add)
        for m in range(2, 8):
            nc.vector.tensor_tensor(
                out=gl[:], in0=gl[:],
                in1=slot_st[phase][:, 2 * (m - 1):2 * m], op=AO.add)
        return gl

    # ------------------------------------------------------------------
    # embed: h = relu(x @ We + be)   (feature-major)
    # ------------------------------------------------------------------
    nc.sync.dma_start(xt_sb[:], t_xT[:])
    for t in range(TPC):
        mmp = (ps_b if t % 2 == 0 else ps_a).tile(
            [128, 128], F32, tag="mmp", name="mmp_e")
        nc.tensor.matmul(out=mmp[:], lhsT=We_sb[:], rhs=xt_sb[:, ts(t)],
                         start=True, stop=True)
        if t % 2 == 0:
            nc.scalar.activation(h_fm[:, ts(t)], mmp[:], AF.Relu,
                                 bias=be_sb[:])
        else:
            nc.vector.tensor_scalar(out=h_fm[:, ts(t)], in0=mmp[:],
                                    scalar1=be_sb[:], scalar2=None,
                                    op0=AO.add)
            nc.vector.tensor_scalar_max(h_fm[:, ts(t)], h_fm[:, ts(t)], 0.0)
    for t in range(TPC):
        mm2 = (ps_a if t % 2 == 0 else ps_b).tile(
            [128, 128], F32, tag="mmp", name="mmp2_e")
        nc.tensor.matmul(out=mm2[:], lhsT=h_fm[:, ts(t)], rhs=W_sb[1][:],
                         start=True, stop=True)
        if t % 2 == 0:
            nc.scalar.activation(tshard_sb[:, ts(t)], mm2[:], AF.Copy,
                                 scale=dinv_sb[:, t:t + 1])
        else:
            nc.vector.tensor_scalar(out=tshard_sb[:, ts(t)], in0=mm2[:],
                                    scalar1=dinv_sb[:, t:t + 1], scalar2=None,
                                    op0=AO.mult)
    flush_tshard(tables[1])

    # ------------------------------------------------------------------
    # 3 GCN blocks
    # ------------------------------------------------------------------
    for layer in (1, 2, 3):
        sum_ps = ps_st.tile([128, 1], F32, tag="sum", name=f"sum{layer}")
        sq_ps = ps_st.tile([128, 1], F32, tag="sq", name=f"sq{layer}")
        for batch in plan["batches"]:
            nch = batch["nchunks"]
            c0 = batch["chunk0"]
            idxb = sb.tile([128, nch * 8], I16, tag="idxb", name="idxb",
                           padded_shape=[128, plan["maxnch"] * 8])
            nc.sync.dma_start(idxb[:], t_idx[:, c0 * 8:(c0 + nch) * 8])
            drelb = sb.tile([128, nch], F16, tag="drelb", name="drelb",
                            padded_shape=[128, plan["maxnch"]])
            nc.sync.dma_start(drelb[:], t_drel[:, c0:c0 + nch])
            aggp = {wl: ps_agg.tile([128, 128], F32, tag=f"aggp{wl}",
                                    name=f"aggp{wl}")
                    for wl in range(len(batch["wins"]))}
            for call in batch["calls"]:
                k, g, gid0 = call["k"], call["g"], call["gid0"]
                lc = gid0 - c0
                msg = msgp.tile([128, k * 128], F16, tag="msg", name="msg",
                                padded_shape=[128, KMAX * 128])
                cid = call["cid"]
                nc.gpsimd.reg_load(ncnt_reg, ncnt_sb[0:1, cid:cid + 1])
                nc.gpsimd.dma_gather(
                    out_ap=msg[:].rearrange("p (c e) -> p c e", e=HID),
                    in_ap=tables[layer][GBASES[g]:GBASES[g] + GSIZES[g], :],
                    idxs_ap=idxb[:, lc * 8:(lc + k) * 8],
                    num_idxs=k * 128, num_idxs_reg=ncnt_reg, elem_size=HID,
                    single_packet=False)
                scall = sb.tile([128, k * 128], F16, tag="scall", name="scall",
                                padded_shape=[128, KMAX * 128])
                drs = drelb[:, lc:lc + k]
                in0 = AP(iota_f[:].tensor, iota_f[:].offset,
                         [iota_f[:].ap[0], [0, k], iota_f[:].ap[1]])
                in1 = AP(drs.tensor, drs.offset,
                         [drs.ap[0], drs.ap[1], [0, 128]])
                nc.vector.tensor_tensor(
                    out=scall[:].rearrange("p (c e) -> p c e", e=128),
                    in0=in0, in1=in1, op=AO.is_equal)
                for j, (w, st, sp) in enumerate(call["chunks"]):
                    wl = w % WINB
                    nc.tensor.matmul(
                        out=aggp[wl][:],
                        lhsT=scall[:, j * 128:(j + 1) * 128],
                        rhs=msg[:, j * 128:(j + 1) * 128],
                        start=st, stop=sp)
            for w in batch["wins"]:
                wl = w % WINB
                # self-loop term: ident @ tshard adds dinv*(h@W) into the
                # window's PSUM; the evac's dinv scale makes it dinv^2*(h@W).
                # This matmul also closes the accumulation group (stop).
                nc.tensor.matmul(out=aggp[wl][:], lhsT=ident[:],
                                 rhs=tshard_sb[:, ts(w)],
                                 start=False, stop=True)
                nc.scalar.activation(agg_big[:, ts(w)], aggp[wl][:],
                                     AF.Copy, scale=dinv_sb[:, w:w + 1])
                sq = sb.tile([128, 128], F16, tag="sq", name="sqt")
                nc.vector.tensor_tensor(out=sq[:], in0=agg_big[:, ts(w)],
                                        in1=agg_big[:, ts(w)], op=AO.mult)
                nc.tensor.matmul(out=sum_ps[:], lhsT=agg_big[:, ts(w)],
                                 rhs=ones_col[:],
                                 start=(w == 0), stop=(w == TPC - 1))
                nc.tensor.matmul(out=sq_ps[:], lhsT=sq[:], rhs=ones_col[:],
                                 start=(w == 0), stop=(w == TPC - 1))

        # ---- BN stats exchange ([128, 2] column stats)
        nc.vector.tensor_copy(st_l[layer][:, 0:1], sum_ps[:])
        nc.vector.tensor_copy(st_l[layer][:, 1:2], sq_ps[:])
        gl_sb = exchange_stats(layer)

        # ---- BN affine coefficients A, B [128, 1]
        stat = sb.tile([128, 6], F32, tag="bn", name="bn")
        mu, ex2, var, rs, A, B = (stat[:, i:i + 1] for i in range(6))
        nc.vector.tensor_scalar_mul(mu, gl_sb[:, 0:1], 1.0 / N_NODES)
        nc.vector.tensor_scalar_mul(ex2, gl_sb[:, 1:2], 1.0 / N_NODES)
        nc.vector.tensor_tensor(out=var, in0=mu, in1=mu, op=AO.mult)
        nc.vector.tensor_tensor(out=var, in0=ex2, in1=var, op=AO.subtract)
        sd = sb.tile([128, 1], F32, tag="sd", name="sd")
        nc.scalar.activation(sd[:], var, AF.Sqrt, bias=eps_col[:])
        nc.vector.reciprocal(rs, sd[:])
        nc.vector.tensor_tensor(out=A, in0=rs, in1=g_sb[layer][:], op=AO.mult)
        nc.vector.tensor_tensor(out=B, in0=mu, in1=A, op=AO.mult)
        nc.vector.tensor_tensor(out=B, in0=bt_sb[layer][:], in1=B,
                                op=AO.subtract)

        # ---- h += relu(A*agg + B)   (transpose to feature-major, fused BN)
        Wn = W_sb[layer + 1] if layer < 3 else None
        for t in range(TPC):
            trp = ps_agg.tile([128, 128], F16, tag=f"aggp{t % WINB}",
                              name=f"trp{t % WINB}")
            nc.tensor.transpose(out=trp[:], in_=agg_big[:, ts(t)],
                                identity=ident[:])
            y = sb.tile([128, 128], F16, tag="y", name="y")
            if t % 2 == 0:
                nc.scalar.activation(y[:], trp[:], AF.Relu, scale=A, bias=B)
            else:
                # split the BN load across ACT and DVE
                nc.vector.tensor_scalar(out=y[:], in0=trp[:], scalar1=A,
                                        scalar2=B, op0=AO.mult, op1=AO.add)
                nc.vector.tensor_scalar_max(y[:], y[:], 0.0)
            nc.vector.tensor_tensor(out=h_fm[:, ts(t)], in0=y[:],
                                    in1=h_fm[:, ts(t)], op=AO.add)
        if Wn is not None:
            for t in range(TPC):
                mmp = (ps_b if t % 2 == 0 else ps_a).tile(
                    [128, 128], F32, tag="mmp", name="mmp")
                nc.tensor.matmul(out=mmp[:], lhsT=h_fm[:, ts(t)], rhs=Wn[:],
                                 start=True, stop=True)
                if t % 2 == 0:
                    nc.scalar.activation(tshard_sb[:, ts(t)], mmp[:], AF.Copy,
                                         scale=dinv_sb[:, t:t + 1])
                else:
                    nc.vector.tensor_scalar(
                        out=tshard_sb[:, ts(t)], in0=mmp[:],
                        scalar1=dinv_sb[:, t:t + 1], scalar2=None,
                        op0=AO.mult)
            flush_tshard(tables[layer + 1])

    # ------------------------------------------------------------------
    # head: out = tanh(relu(BN(h3 @ Wf1)) @ Wf2 + bf2)
    # ------------------------------------------------------------------
    fsum_ps = ps_st.tile([32, 1], F32, tag="sum", name="fsum")
    fsq_ps = ps_st.tile([32, 1], F32, tag="sq", name="fsq")
    for t in range(TPC):
        fp = (ps_b if t % 2 == 0 else ps_a).tile(
            [128, 32], F32, tag="mmp", name="fp")
        nc.tensor.matmul(out=fp[:], lhsT=h_fm[:, ts(t)], rhs=Wf1_sb[:],
                         start=True, stop=True)
        fs = slice(t * 32, (t + 1) * 32)
        nc.vector.tensor_scalar(out=fpre_big[:, fs], in0=fp[:],
                                scalar1=msk_sb[:, t:t + 1], scalar2=None,
                                op0=AO.mult)
    for t in range(TPC):
        fs = slice(t * 32, (t + 1) * 32)
        sq = sb.tile([128, 32], F16, tag="sq32", name="sq32")
        nc.vector.tensor_tensor(out=sq[:], in0=fpre_big[:, fs],
                                in1=fpre_big[:, fs], op=AO.mult)
        nc.tensor.matmul(out=fsum_ps[:], lhsT=fpre_big[:, fs], rhs=ones_col[:],
                         start=(t == 0), stop=(t == TPC - 1))
        nc.tensor.matmul(out=fsq_ps[:], lhsT=sq[:], rhs=ones_col[:],
                         start=(t == 0), stop=(t == TPC - 1))

    nc.vector.tensor_copy(st_l[4][:32, 0:1], fsum_ps[:])
    nc.vector.tensor_copy(st_l[4][:32, 1:2], fsq_ps[:])
    fgl = exchange_stats(4)

    fstat = sb.tile([32, 6], F32, tag="bn", name="fbn")
    mu, ex2, var, rs, A, B = (fstat[:, i:i + 1] for i in range(6))
    nc.vector.tensor_scalar_mul(mu, fgl[:32, 0:1], 1.0 / N_NODES)
    nc.vector.tensor_scalar_mul(ex2, fgl[:32, 1:2], 1.0 / N_NODES)
    nc.vector.tensor_tensor(out=var, in0=mu, in1=mu, op=AO.mult)
    nc.vector.tensor_tensor(out=var, in0=ex2, in1=var, op=AO.subtract)
    fsd = sb.tile([32, 1], F32, tag="sd", name="fsd")
    nc.scalar.activation(fsd[:], var, AF.Sqrt, bias=eps_col[:32, :])
    nc.vector.reciprocal(rs, fsd[:])
    nc.vector.tensor_tensor(out=A, in0=rs, in1=gf_sb[:], op=AO.mult)
    nc.vector.tensor_tensor(out=B, in0=mu, in1=A, op=AO.mult)
    nc.vector.tensor_tensor(out=B, in0=btf_sb[:], in1=B, op=AO.subtract)

    for t in range(TPC):
        fs = slice(t * 32, (t + 1) * 32)
        trf = ps_agg.tile([32, 128], F16, tag=f"aggp{t % WINB}",
                          name=f"trf{t % WINB}")
        nc.tensor.transpose(out=trf[:], in_=fpre_big[:, fs], identity=ident[:])
        f_fm = sb.tile([33, 128], F16, tag="f", name="f")
        if t % 2 == 0:
            nc.scalar.activation(f_fm[:32, :], trf[:], AF.Relu,
                                 scale=A, bias=B)
        else:
            nc.vector.tensor_scalar(out=f_fm[:32, :], in0=trf[:], scalar1=A,
                                    scalar2=B, op0=AO.mult, op1=AO.add)
            nc.vector.tensor_scalar_max(f_fm[:32, :], f_fm[:32, :], 0.0)
        nc.vector.memset(f_fm[32:33, :], 1.0)
        op = (ps_b if t % 2 == 0 else ps_a).tile(
            [128, 2], F32, tag="mmp", name="op")
        nc.tensor.matmul(out=op[:], lhsT=f_fm[:], rhs=Wf2_sb[:],
                         start=True, stop=True)
        nc.scalar.activation(out_nm[:, 2 * t:2 * t + 2], op[:], AF.Tanh)
    nc.sync.dma_start(t_out[:], out_nm[:])


# ----------------------------------------------------------------------------
# Public entry point
# ----------------------------------------------------------------------------

_CACHE = {}

_PREP_VERSION = "v6_slmm"


def _get_compiled(edge_index, x):
    key = hash((edge_index.tobytes(), x.shape))
    if key not in _CACHE:
        import os
        plan = streams = None
        cpath = None
        if os.environ.get("KERNEL_DEV_CACHE"):
            import pickle
            cpath = f"/tmp/prep_{_PREP_VERSION}_{key & 0xFFFFFFFF:x}.pkl"
            if os.path.exists(cpath):
                with open(cpath, "rb") as f:
                    plan, streams = pickle.load(f)
        if plan is None:
            plan, streams = _preprocess(edge_index, x)
            if cpath:
                import pickle
                with open(cpath, "wb") as f:
                    pickle.dump((plan, streams), f)
        nc = _build_program(plan)
        _CACHE.clear()
        _CACHE[key] = (nc, streams)
    return _CACHE[key]


def _in_maps(streams, kw):
    rep = dict(
        We=np.asarray(kw["We"], np.float16),
        W1=np.asarray(kw["W1"], np.float16),
        W2=np.asarray(kw["W2"], np.float16),
        W3=np.asarray(kw["W3"], np.float16),
        Wf1=np.asarray(kw["Wf1"], np.float16),
        Wf2e=np.concatenate(
            [np.asarray(kw["Wf2"], np.float32),
             np.asarray(kw["bf2"], np.float32)[None, :]], 0
        ).astype(np.float16),
        be_col=np.asarray(kw["be"], np.float32)[:, None],
        g1=np.asarray(kw["g1"], np.float32)[:, None],
        bt1=np.asarray(kw["bt1"], np.float32)[:, None],
        g2=np.asarray(kw["g2"], np.float32)[:, None],
        bt2=np.asarray(kw["bt2"], np.float32)[:, None],
        g3=np.asarray(kw["g3"], np.float32)[:, None],
        bt3=np.asarray(kw["bt3"], np.float32)[:, None],
        gf=np.asarray(kw["gf"], np.float32)[:, None],
        btf=np.asarray(kw["btf"], np.float32)[:, None],
    )
    return [dict(rep, **streams[c]) for c in range(NCORES)]


def run(trace=False, tmpdir=None, **kw):
    x = np.asarray(kw["x"], np.float32)
    edge_index = np.asarray(kw["edge_index"], np.int32)
    nc, streams = _get_compiled(edge_index, x)
    res = run_bass_kernel_spmd(nc, _in_maps(streams, kw),
                               core_ids=list(range(NCORES)), trace=trace,
                               tmpdir=tmpdir)
    shards = []
    for c in range(NCORES):
        buf = res.results[c]["out"]                   # [128, TPC*2]
        nm = buf.reshape(128, TPC, 2).transpose(1, 0, 2).reshape(NPAD, 2)
        shards.append(nm[:NSH])
    out = np.ascontiguousarray(np.concatenate(shards, 0))
    return out, res


def kernel(**kw):
    out, _ = run(trace=False, **kw)
    return out



# revision 13
# speedup vs baseline: 1.4544x; 1.4544x over previous
"""Trainium2 Bass kernel for nn_EnhancedGNN (3-block GCN + BN + MLP head).

Strategy (8 NeuronCores, node-partitioned graph parallel):
  - Nodes are partitioned contiguously across the 8 cores (12500 each,
    padded to 12544 = 98*128). Each core owns all edges whose dst lies in
    its partition. Self-loops never enter the message stream: their
    contribution (dinv^2 * h@W) is added per window from the local shard.
  - Per GCN layer, each core holds a full fp16 feature table
    H~ = dinv * (h @ W) in its DRAM, replicated NOT via collectives but by
    an XOR-pair RDMA exchange: step m swaps half-shards with core (own^m)
    through SBUF slots (remote_dma_broadcast), paced by per-round ack
    semaphores; cross-die dests carry a ^2 routing compensation. Each
    core's table stores shard s at region (s ^ own), row-interleaved so
    drains write contiguous 25KB runs. Messages are fetched with
    dma_gather (random 256B rows, int16 group-relative indices, trailing
    pads negative + per-core valid counts in a register stream).
  - Segment-sum by dst is a one-hot matmul: for each chunk of 128
    dst-sorted messages, S[msg, dstslot] = (dstrel[msg] == iota[slot])
    built on DVE, then PSUM += S^T @ msgs on TensorE. dinv[dst] is applied
    on PSUM evacuation; dinv[src] is folded into the table rows; the GCN
    bias is absorbed by the following BatchNorm.
  - h lives FEATURE-major ([feat, node]) so the h @ W table matmuls need
    no transpose, and BN+ReLU fuses into one per-partition scale/bias op
    (alternating between ACT and DVE to balance engine load). BN stats
    are column vectors from ones-matmuls, summed across cores by tiny
    RDMA broadcasts with per-phase semaphores.
"""

import numpy as np

import concourse.bacc as bacc
import concourse.mybir as mybir
import concourse.tile as tile
from concourse.bass import AP
from concourse.bass_utils import run_bass_kernel_spmd
from concourse.masks import make_identity

F32 = mybir.dt.float32
F16 = mybir.dt.float16
I16 = mybir.dt.int16
I32 = mybir.dt.int32
I8 = mybir.dt.int8

N_NODES = 100000
HID = 128
NCORES = 8
EPS = 1e-5
RG = [list(range(NCORES))]
WINB = 4                         # windows per PSUM batch ([128, 512] tile)
KMAX = 32                        # chunks per dma_gather call


def _set_config(n_nodes=100000, group=32768):
    """Derive all sharding constants (module globals) from the node count."""
    global N_NODES, NSH, TPC, NPAD, TROWS, GROUP, GBASES, GSIZES, NG
    N_NODES = n_nodes
    NSH = N_NODES // NCORES
    TPC = (NSH + 127) // 128
    NPAD = TPC * 128
    TROWS = NCORES * NPAD
    GROUP = group
    GBASES = list(range(0, TROWS, GROUP))
    GSIZES = [min(GROUP, TROWS - b) for b in GBASES]
    NG = len(GBASES)


_set_config()


# ----------------------------------------------------------------------------
# Host-side preprocessing: edge bucketing + per-core streams
# ----------------------------------------------------------------------------

def _preprocess(edge_index, x):
    # Self-loops are excluded from the gather stream: their source rows are
    # the core's own tshard_sb (already in SBUF) and are added on-device as
    # dinv * tshard per window. Degrees still count the loops.
    src = edge_index[0].astype(np.int64)
    dst = edge_index[1].astype(np.int64)

    deg = (np.bincount(dst, minlength=N_NODES) + 1).astype(np.float32)
    dinv = (1.0 / np.sqrt(deg)).astype(np.float32)

    owner = src // NSH
    loc = src % NSH
    # XOR-permuted region + row-interleaved layout: on core c, the shard of
    # core s lives at table rows [(s ^ c) * NPAD, ...), row = (loc%128)*TPC
    # + loc//128 so the RDMA drain writes contiguous 25KB runs per partition.
    rowin = (loc % 128) * TPC + (loc // 128)
    core = dst // NSH
    dloc = dst % NSH
    win = dloc // 128
    drel = dloc % 128

    counts = np.zeros((NCORES, TPC, NG), np.int64)
    percore = []
    for c in range(NCORES):
        m = core == c
        s_c = (owner[m] ^ c) * NPAD + rowin[m]   # per-core table row id
        g_c = s_c // GROUP
        w_c, d_c = win[m], drel[m]
        o = np.lexsort((d_c, g_c, w_c))
        s_c, w_c, g_c, d_c = s_c[o], w_c[o], g_c[o], d_c[o]
        cnt = np.zeros((TPC, NG), np.int64)
        np.add.at(cnt, (w_c, g_c), 1)
        counts[c] = cnt
        percore.append((s_c, d_c, cnt))
    cmax = counts.max(0)
    chunks = -(-cmax // 128)                     # [TPC, NG] ceil
    assert (chunks.sum(1) > 0).all()

    # ---- static call plan shared by every core
    win_total = chunks.sum(1)
    batches = []
    bucket_gids = {}                             # (w, g) -> [chunk gids]
    gid = 0
    ncalls = 0
    nb = -(-TPC // WINB)
    for b in range(nb):
        wins = list(range(b * WINB, min((b + 1) * WINB, TPC)))
        emitted = {w: 0 for w in wins}
        calls = []
        chunk0 = gid
        for g in range(NG):
            pend = []
            for w in wins:
                if chunks[w, g]:
                    bucket_gids.setdefault((w, g), [])
                    pend.extend([w] * chunks[w, g])
            for i0 in range(0, len(pend), KMAX):
                sub = pend[i0:i0 + KMAX]
                ch = []
                for j, w in enumerate(sub):
                    st = emitted[w] == 0
                    emitted[w] += 1
                    # stop is carried by the self-loop identity matmul that
                    # closes each window's accumulation in the evac section
                    ch.append((w, st, False))
                    bucket_gids[(w, g)].append(gid + j)
                calls.append(dict(g=g, k=len(sub), chunks=ch, gid0=gid,
                                  cid=ncalls))
                ncalls += 1
                gid += len(sub)
        batches.append(dict(wins=wins, calls=calls, chunk0=chunk0,
                            nchunks=gid - chunk0))
    tot_chunks = gid

    # ---- per-core streams
    streams = []
    for c in range(NCORES):
        s_c, d_c, cnt = percore[c]
        # offsets of each (w, g) bucket in the lexsorted arrays
        flat = cnt.reshape(-1)
        offs = np.concatenate([[0], np.cumsum(flat)[:-1]]).reshape(TPC, NG)
        # pad slots carry idx -1: the gather ucode skips negative indices,
        # saving the padding fetch bandwidth (S is zero there regardless)
        idx_stream = np.full((tot_chunks, 128), -1, np.int16)
        drel_stream = np.full((tot_chunks, 128), -1.0, np.float32)
        for (w, g), gids in bucket_gids.items():
            n = cnt[w, g]
            ncap = len(gids) * 128
            o0 = offs[w, g]
            vals = np.full(ncap, -1, np.int64)
            vals[:n] = s_c[o0:o0 + n] - GBASES[g]
            dr = np.full(ncap, -1.0, np.float32)
            dr[:n] = d_c[o0:o0 + n]
            idx_stream[gids] = vals.reshape(-1, 128).astype(np.int16)
            drel_stream[gids] = dr.reshape(-1, 128)
        # Reorder each call's chunks so bucket-tail (likely-partial) chunks
        # land at the call's end where the trailing-negative skip (ncnt)
        # can cut them. Shared across cores (same static plan); applied to
        # core 0's pass only via reorder_done guard below.
        if c == 0:
            for b in batches:
                for call in b["calls"]:
                    k, g0, g = call["k"], call["gid0"], call["g"]
                    rank = []
                    for j, (w, _st, sp) in enumerate(call["chunks"]):
                        gl = bucket_gids[(w, g)]
                        rank.append(len(gl) - 1 - gl.index(g0 + j))
                    perm = sorted(range(k), key=lambda j: (-rank[j], j))
                    call["perm"] = perm
            # start flag goes to each window's first chunk in execution order
            for b in batches:
                seen = set()
                for call in b["calls"]:
                    ch = [call["chunks"][j] for j in call["perm"]]
                    newch = []
                    for (w, _st, sp) in ch:
                        newch.append((w, w not in seen, sp))
                        seen.add(w)
                    call["chunks"] = newch
        for b in batches:
            for call in b["calls"]:
                k, g0 = call["k"], call["gid0"]
                perm = call["perm"]
                idx_stream[g0:g0 + k] = idx_stream[[g0 + j for j in perm]]
                drel_stream[g0:g0 + k] = drel_stream[[g0 + j for j in perm]]
        # Negative idxs are skipped by the gather ucode, but only as a
        # trailing suffix: keep -1 for each call's tail pads, remap interior
        # pads to dummy row 0, and pass count = last valid + 1 per call.
        ncnt = np.zeros((1, ncalls), np.int32)
        for b in batches:
            for call in b["calls"]:
                g0, k = call["gid0"], call["k"]
                blk = idx_stream[g0:g0 + k].reshape(-1)
                nz = np.nonzero(blk >= 0)[0]
                L = int(nz[-1]) + 1 if len(nz) else 0
                head = blk[:L]
                head[head < 0] = 0
                blk[:L] = head
                idx_stream[g0:g0 + k] = blk.reshape(k, 128)
                ncnt[0, call["cid"]] = L
        # wrap indices: chunk j, msg p -> [p % 16, j*8 + p//16]; the x8
        # partition replication the gather ucode wants is done on-device
        idx16 = np.ascontiguousarray(
            idx_stream.reshape(tot_chunks * 8, 16).T)             # [16, 8*T]
        drel_cols = np.ascontiguousarray(drel_stream.T).astype(np.int8)
        dv = np.zeros(NPAD, np.float32)
        dv[:NSH] = dinv[c * NSH:(c + 1) * NSH]
        dinv_cols = np.ascontiguousarray(dv.reshape(TPC, 128).T)  # [128, TPC]
        mk = np.zeros(NPAD, np.float32)
        mk[:NSH] = 1.0
        msk_cols = np.ascontiguousarray(mk.reshape(TPC, 128).T)
        xp = np.zeros((NPAD, 2), np.float32)
        xp[:NSH] = x[c * NSH:(c + 1) * NSH]
        xT = np.ascontiguousarray(xp.T).astype(np.float16)        # [2, NPAD]
        streams.append(dict(idxs=idx16, drel=drel_cols,
                            dinv=dinv_cols, msk=msk_cols, xT=xT, ncnt=ncnt))

    plan = dict(batches=batches, tot_chunks=tot_chunks,
                tot_cols=tot_chunks * 8, ncalls=ncalls,
                maxnch=max(b["nchunks"] for b in batches))
    return plan, streams


# ----------------------------------------------------------------------------
# Device program
# ----------------------------------------------------------------------------

NQ = 4                           # SWDGE queues: gathers on 0..NQ-2, RDMA on NQ-1


def _build_program(plan):
    nc = bacc.Bacc("TRN2", target_bir_lowering=False, debug=False,
                   enable_asserts=True, num_devices=NCORES,
                   num_swdge_queues=NQ)

    def din(name, shape, dt=F32):
        return nc.dram_tensor(name, list(shape), dt, kind="ExternalInput").ap()

    t_idx = din("idxs", [16, plan["tot_cols"]], I16)
    t_ncnt = din("ncnt", [1, plan["ncalls"]], I32)
    t_drel = din("drel", [128, plan["tot_chunks"]], I8)
    t_dinv = din("dinv", [128, TPC])
    t_msk = din("msk", [128, TPC])
    t_xT = din("xT", [2, NPAD], F16)
    t_We = din("We", [2, HID], F16)
    t_W = {1: din("W1", [HID, HID], F16), 2: din("W2", [HID, HID], F16),
           3: din("W3", [HID, HID], F16)}
    t_Wf1 = din("Wf1", [HID, 32], F16)
    t_Wf2 = din("Wf2e", [33, 2], F16)
    t_be = din("be_col", [HID, 1])
    t_g = {i: din(f"g{i}", [HID, 1]) for i in (1, 2, 3)}
    t_bt = {i: din(f"bt{i}", [HID, 1]) for i in (1, 2, 3)}
    t_gf = din("gf", [32, 1])
    t_btf = din("btf", [32, 1])
    t_out = nc.dram_tensor("out", [128, TPC * 2], F32,
                           kind="ExternalOutput").ap()

    from contextlib import ExitStack
    with tile.TileContext(nc) as tc, ExitStack() as st:
        cst = st.enter_context(tc.tile_pool(name="cst", bufs=1))
        sb = st.enter_context(tc.tile_pool(name="sb", bufs=2))
        msgp = st.enter_context(tc.tile_pool(name="msgp", bufs=4))
        ps_agg = st.enter_context(tc.tile_pool(name="ps_agg", bufs=1, space="PSUM"))
        ps_st = st.enter_context(tc.tile_pool(name="ps_st", bufs=1, space="PSUM"))
        ps_a = st.enter_context(tc.tile_pool(name="ps_a", bufs=1, space="PSUM"))
        ps_b = st.enter_context(tc.tile_pool(name="ps_b", bufs=1, space="PSUM"))
        dr = st.enter_context(tc.tile_pool(name="dr", bufs=1, space="DRAM"))
        _emit(nc, tc, plan, locals())
    nc.compile()
    return nc


def _emit(nc, tc, plan, pools):
    cst, sb, msgp = pools["cst"], pools["sb"], pools["msgp"]
    ps_agg, ps_st = pools["ps_agg"], pools["ps_st"]
    ps_a, ps_b, dr = pools["ps_a"], pools["ps_b"], pools["dr"]
    t_idx, t_drel = pools["t_idx"], pools["t_drel"]
    t_ncnt = pools["t_ncnt"]
    t_dinv, t_xT, t_We = pools["t_dinv"], pools["t_xT"], pools["t_We"]
    t_W, t_Wf1, t_Wf2 = pools["t_W"], pools["t_Wf1"], pools["t_Wf2"]
    t_be, t_g, t_bt = pools["t_be"], pools["t_g"], pools["t_bt"]
    t_gf, t_btf = pools["t_gf"], pools["t_btf"]
    t_out = pools["t_out"]
    AO, AF = mybir.AluOpType, mybir.ActivationFunctionType

    # ---- constants
    iota_i = cst.tile([128, 128], I32)
    nc.gpsimd.iota(iota_i[:], pattern=[[1, 128]], base=0, channel_multiplier=0)
    iota_f32 = cst.tile([128, 128], F32)
    nc.vector.tensor_copy(iota_f32[:], iota_i[:])
    iota_f = cst.tile([128, 128], F16)
    nc.vector.tensor_copy(iota_f[:], iota_f32[:])
    ident = cst.tile([128, 128], F16)
    make_identity(nc, ident[:])
    ones_col = cst.tile([128, 1], F16)
    nc.vector.memset(ones_col[:], 1.0)
    eps_col = cst.tile([128, 1], F32)
    nc.vector.memset(eps_col[:], EPS)

    def load_const(t, shape, dt=F32):
        tl = cst.tile(shape, dt, name=f"c_{t.tensor.name}")
        nc.sync.dma_start(tl[:], t[:])
        return tl

    dinv_sb = load_const(t_dinv, [128, TPC])
    ncnt_sb = load_const(t_ncnt, [1, plan["ncalls"]], I32)
    ncnt_reg = nc.gpsimd.alloc_register("ncnt_reg")

    # idx stream: uploaded once as [16, 8T]; the x8 partition replication
    # the gather ucode expects is materialized by 8 strided DMAs here.
    idxb_all = cst.tile([128, plan["tot_cols"]], I16)
    for r in range(8):
        nc.sync.dma_start(idxb_all[16 * r:16 * (r + 1), :], t_idx[:])
    drel_i8 = cst.tile([128, plan["tot_chunks"]], I8)
    nc.sync.dma_start(drel_i8[:], t_drel[:])
    drel_all = cst.tile([128, plan["tot_chunks"]], F16)
    nc.vector.tensor_copy(drel_all[:], drel_i8[:])
    msk_sb = load_const(pools["t_msk"], [128, TPC])
    We_sb = load_const(t_We, [2, HID], F16)
    W_sb = {i: load_const(t_W[i], [HID, HID], F16) for i in (1, 2, 3)}
    Wf1_sb = load_const(t_Wf1, [HID, 32], F16)
    Wf2_sb = load_const(t_Wf2, [33, 2], F16)
    be_sb = load_const(t_be, [HID, 1])
    g_sb = {i: load_const(t_g[i], [HID, 1]) for i in (1, 2, 3)}
    bt_sb = {i: load_const(t_bt[i], [HID, 1]) for i in (1, 2, 3)}
    gf_sb = load_const(t_gf, [32, 1])
    btf_sb = load_const(t_btf, [32, 1])

    h_fm = cst.tile([128, NPAD], F16)        # FEATURE-major h [feat, node]
    agg_big = cst.tile([128, NPAD], F16)     # node-major aggregation output
    fpre_big = cst.tile([128, TPC * 32], F16)
    tshard_sb = cst.tile([128, NPAD], F16)   # staging for the table shard
    out_nm = cst.tile([128, TPC * 2], F32)   # node-major staged output
    HTPC = TPC // 2
    slot_t = [cst.tile([128, HTPC * 128], F16, name=f"slot{p}")
              for p in (0, 1)]
    st_l = {i: cst.tile([128, 2], F32, name=f"st{i}") for i in (1, 2, 3, 4)}
    slot_st = {i: cst.tile([128, 14], F32, name=f"sst{i}")
               for i in (1, 2, 3, 4)}

    tables = {i: dr.tile([TROWS, HID], F16, name=f"table{i}")
              for i in (1, 2, 3)}

    import os
    SIM_STUB = bool(os.environ.get("KERNEL_SIM_STUB"))

    # XOR-pair exchange: step m swaps shards with core (own ^ m). Cross-die
    # dests (bit 2) get an extra ^2 from the ucode lane balance; compensate
    # (the interpreter models ideal routing — KERNEL_RDMA_NOCOMP disables it).
    NOCOMP = bool(os.environ.get("KERNEL_RDMA_NOCOMP"))

    def rd(m):
        if NOCOMP:
            return (0, m)
        return (0, m ^ 2 if m >= 4 else m)

    RD1 = {m: [None] * 8 for m in range(1, 8)}
    for m in range(1, 8):
        RD1[m][m] = rd(m)
    RD_ALL = [None] + [rd(k) for k in range(1, 8)]

    nc.vector.memset(st_l[4][:], 0.0)
    for _ in range(4):
        # initialize all three rotating gather buffers: rows of slots the
        # gather skips (trailing pads) are read by the matmul (times S=0)
        mz = msgp.tile([128, KMAX * 128], F16, tag="msg", name="msg")
        nc.vector.memset(mz[:], 0.0)

    rsem_t = [nc.alloc_semaphore("rsem_t0"), nc.alloc_semaphore("rsem_t1")]
    rsem_s = {i: nc.alloc_semaphore(f"rsem_s{i}") for i in (1, 2, 3, 4)}
    lsem_t = nc.alloc_semaphore("lsem_t")
    lsem_s = nc.alloc_semaphore("lsem_s")
    ack_sems = [nc.alloc_semaphore(f"ack{j}") for j in range(14)]
    ack_lsem = nc.alloc_semaphore("ack_lsem")
    psem = nc.alloc_semaphore("psem")
    tsem = nc.alloc_semaphore("tsem")
    cnt = dict(prep=0, send_t=0, arr0=0, arr1=0, drain=0, flush=0)

    def prep_trig(inst):
        inst.then_inc(psem, 1)
        cnt["prep"] += 1
        nc.gpsimd.wait_ge(psem, cnt["prep"])
        nc.gpsimd.trigger_dma(count=1, queue_num=NQ - 1)

    def ts(t):
        return slice(t * 128, (t + 1) * 128)

    def region(table, m):
        return table[m * NPAD:(m + 1) * NPAD, :].rearrange(
            "(p w) f -> p w f", w=TPC)

    def flush_tshard(table):
        """XOR-pair RDMA exchange of tshard_sb into the local table copy.

        Half-shard rounds: round r = (m-1)*2 + h sends window-half h of the
        shard to peer (own ^ m), 2-slot pipelined with a global drain-ack
        barrier gating slot reuse (round r waits acks of rounds <= r-2)."""
        if SIM_STUB:
            # Cost-sim stand-in: the local drains only (the broadcasts and
            # handshake can't be modeled by a single-core timeline sim).
            for m in range(8):
                nc.sync.dma_start(
                    region(table, m),
                    tshard_sb[:].rearrange("p (w f) -> p w f", f=HID))
            return
        F = cnt["flush"]
        cnt["flush"] += 1
        with tc.tile_critical():
            nc.sync.dma_start(
                region(table, 0),
                tshard_sb[:].rearrange("p (w f) -> p w f", f=HID)
            ).then_inc(tsem, 16)
            cnt["drain"] += 16
            for m in range(1, 8):
                for h in (0, 1):
                    j = (m - 1) * 2 + h          # round within this flush
                    # slot-reuse barrier: round j-2 (same flush; or j+12 of
                    # the previous flush) drained on every core
                    if not SIM_STUB and (F > 0 or j >= 2):
                        if j >= 2:
                            nc.gpsimd.wait_ge(ack_sems[j - 2], 14 * (F + 1))
                        else:
                            nc.gpsimd.wait_ge(ack_sems[j + 12], 14 * F)
                    cs = slice(h * HTPC * 128, (h + 1) * HTPC * 128)
                    prep_trig(nc.gpsimd.remote_dma_broadcast(
                        out_ap=slot_t[j % 2][:], in_ap=tshard_sb[:, cs],
                        remote_sem=rsem_t[j % 2], local_sem=lsem_t,
                        rdests=RD1[m], queue_num=NQ - 1))
                    cnt["send_t"] += 16
                    if j > 0:
                        # ack round j-1 (its drain was issued last iteration)
                        nc.gpsimd.wait_ge(tsem, cnt["drain"])
                        prep_trig(nc.gpsimd.remote_sem_update_broadcast(
                            ack_sems[j - 1], ack_lsem, rdests=RD_ALL,
                            queue_num=NQ - 1))
                    par = j % 2
                    key = "arr0" if par == 0 else "arr1"
                    cnt[key] += 2
                    if not SIM_STUB:
                        nc.sync.wait_ge(rsem_t[par], cnt[key])
                    nc.sync.dma_start(
                        region(table, m)[:, h * HTPC:(h + 1) * HTPC, :],
                        slot_t[par][:].rearrange("p (w f) -> p w f", f=HID)
                    ).then_inc(tsem, 16)
                    cnt["drain"] += 16
            nc.gpsimd.wait_ge(tsem, cnt["drain"])
            prep_trig(nc.gpsimd.remote_sem_update_broadcast(
                ack_sems[13], ack_lsem, rdests=RD_ALL, queue_num=NQ - 1))
            nc.sync.wait_ge(tsem, cnt["drain"])
            nc.gpsimd.wait_ge(lsem_t, cnt["send_t"])

    def exchange_stats(phase):
        """Sum [128, 2] column stats across cores via RDMA broadcasts."""
        gl = sb.tile([128, 2], F32, tag="stv", name=f"glv{phase}")
        if SIM_STUB:
            nc.vector.tensor_copy(gl[:], st_l[phase][:])
            return gl
        with tc.tile_critical():
            for m in range(1, 8):
                prep_trig(nc.gpsimd.remote_dma_broadcast(
                    out_ap=slot_st[phase][:, 2 * (m - 1):2 * m],
                    in_ap=st_l[phase][:], remote_sem=rsem_s[phase],
                    local_sem=lsem_s, rdests=RD1[m], queue_num=NQ - 1))
            if not SIM_STUB:
                nc.vector.wait_ge(rsem_s[phase], 14)
        nc.vector.tensor_tensor(out=gl[:], in0=st_l[phase][:],
                                in1=slot_st[phase][:, 0:2], op=AO.add)
        for m in range(2, 8):
            nc.vector.tensor_tensor(
                out=gl[:], in0=gl[:],
                in1=slot_st[phase][:, 2 * (m - 1):2 * m], op=AO.add)
        return gl

    # ------------------------------------------------------------------
    # embed: h = relu(x @ We + be)   (feature-major)
    # ------------------------------------------------------------------
    for t in range(TPC):
        xt_t = sb.tile([2, 128], F16, tag="xt", name="xt")
        nc.sync.dma_start(xt_t[:], t_xT[:, ts(t)])
        mmp = (ps_b if t % 2 == 0 else ps_a).tile(
            [128, 128], F32, tag="mmp", name="mmp_e")
        nc.tensor.matmul(out=mmp[:], lhsT=We_sb[:], rhs=xt_t[:],
                         start=True, stop=True)
        if t % 2 == 0:
            nc.scalar.activation(h_fm[:, ts(t)], mmp[:], AF.Relu,
                                 bias=be_sb[:])
        else:
            nc.vector.tensor_scalar(out=h_fm[:, ts(t)], in0=mmp[:],
                                    scalar1=be_sb[:], scalar2=None,
                                    op0=AO.add)
            nc.vector.tensor_scalar_max(h_fm[:, ts(t)], h_fm[:, ts(t)], 0.0)
    for t in range(TPC):
        mm2 = (ps_a if t % 2 == 0 else ps_b).tile(
            [128, 128], F32, tag="mmp", name="mmp2_e")
        nc.tensor.matmul(out=mm2[:], lhsT=h_fm[:, ts(t)], rhs=W_sb[1][:],
                         start=True, stop=True)
        if t % 2 == 0:
            nc.scalar.activation(tshard_sb[:, ts(t)], mm2[:], AF.Copy,
                                 scale=dinv_sb[:, t:t + 1])
        else:
            nc.vector.tensor_scalar(out=tshard_sb[:, ts(t)], in0=mm2[:],
                                    scalar1=dinv_sb[:, t:t + 1], scalar2=None,
                                    op0=AO.mult)
    flush_tshard(tables[1])

    # ------------------------------------------------------------------
    # 3 GCN blocks
    # ------------------------------------------------------------------
    for layer in (1, 2, 3):
        sum_ps = ps_st.tile([128, 1], F32, tag="sum", name=f"sum{layer}")
        sq_ps = ps_st.tile([128, 1], F32, tag="sq", name=f"sq{layer}")
        for batch in plan["batches"]:
            aggp = {wl: ps_agg.tile([128, 128], F32, tag=f"aggp{wl}",
                                    name=f"aggp{wl}")
                    for wl in range(len(batch["wins"]))}
            for call in batch["calls"]:
                k, g, gid0 = call["k"], call["g"], call["gid0"]
                msg = msgp.tile([128, k * 128], F16, tag="msg", name="msg",
                                padded_shape=[128, KMAX * 128])
                cid = call["cid"]
                nc.gpsimd.reg_load(ncnt_reg, ncnt_sb[0:1, cid:cid + 1])
                nc.gpsimd.dma_gather(
                    out_ap=msg[:].rearrange("p (c e) -> p c e", e=HID),
                    in_ap=tables[layer][GBASES[g]:GBASES[g] + GSIZES[g], :],
                    idxs_ap=idxb_all[:, gid0 * 8:(gid0 + k) * 8],
                    num_idxs=k * 128, num_idxs_reg=ncnt_reg, elem_size=HID,
                    single_packet=False, queue_num=cid % (NQ - 1))
                scall = sb.tile([128, k * 128], F16, tag="scall", name="scall",
                                padded_shape=[128, KMAX * 128])
                drs = drel_all[:, gid0:gid0 + k]
                in0 = AP(iota_f[:].tensor, iota_f[:].offset,
                         [iota_f[:].ap[0], [0, k], iota_f[:].ap[1]])
                in1 = AP(drs.tensor, drs.offset,
                         [drs.ap[0], drs.ap[1], [0, 128]])
                nc.vector.tensor_tensor(
                    out=scall[:].rearrange("p (c e) -> p c e", e=128),
                    in0=in0, in1=in1, op=AO.is_equal)
                for j, (w, st, sp) in enumerate(call["chunks"]):
                    wl = w % WINB
                    nc.tensor.matmul(
                        out=aggp[wl][:],
                        lhsT=scall[:, j * 128:(j + 1) * 128],
                        rhs=msg[:, j * 128:(j + 1) * 128],
                        start=st, stop=sp)
            for w in batch["wins"]:
                wl = w % WINB
                # self-loop term: ident @ tshard adds dinv*(h@W) into the
                # window's PSUM; the evac's dinv scale makes it dinv^2*(h@W).
                # This matmul also closes the accumulation group (stop).
                nc.tensor.matmul(out=aggp[wl][:], lhsT=ident[:],
                                 rhs=tshard_sb[:, ts(w)],
                                 start=False, stop=True)
                nc.scalar.activation(agg_big[:, ts(w)], aggp[wl][:],
                                     AF.Copy, scale=dinv_sb[:, w:w + 1])
                sq = sb.tile([128, 128], F16, tag="sq", name="sqt")
                nc.vector.tensor_tensor(out=sq[:], in0=agg_big[:, ts(w)],
                                        in1=agg_big[:, ts(w)], op=AO.mult)
                nc.tensor.matmul(out=sum_ps[:], lhsT=agg_big[:, ts(w)],
                                 rhs=ones_col[:],
                                 start=(w == 0), stop=(w == TPC - 1))
                nc.tensor.matmul(out=sq_ps[:], lhsT=sq[:], rhs=ones_col[:],
                                 start=(w == 0), stop=(w == TPC - 1))

        # ---- BN stats exchange ([128, 2] column stats)
        nc.vector.tensor_copy(st_l[layer][:, 0:1], sum_ps[:])
        nc.vector.tensor_copy(st_l[layer][:, 1:2], sq_ps[:])
        gl_sb = exchange_stats(layer)

        # ---- BN affine coefficients A, B [128, 1]
        stat = sb.tile([128, 6], F32, tag="bn", name="bn")
        mu, ex2, var, rs, A, B = (stat[:, i:i + 1] for i in range(6))
        nc.vector.tensor_scalar_mul(mu, gl_sb[:, 0:1], 1.0 / N_NODES)
        nc.vector.tensor_scalar_mul(ex2, gl_sb[:, 1:2], 1.0 / N_NODES)
        nc.vector.tensor_tensor(out=var, in0=mu, in1=mu, op=AO.mult)
        nc.vector.tensor_tensor(out=var, in0=ex2, in1=var, op=AO.subtract)
        sd = sb.tile([128, 1], F32, tag="sd", name="sd")
        nc.scalar.activation(sd[:], var, AF.Sqrt, bias=eps_col[:])
        nc.vector.reciprocal(rs, sd[:])
        nc.vector.tensor_tensor(out=A, in0=rs, in1=g_sb[layer][:], op=AO.mult)
        nc.vector.tensor_tensor(out=B, in0=mu, in1=A, op=AO.mult)
        nc.vector.tensor_tensor(out=B, in0=bt_sb[layer][:], in1=B,
                                op=AO.subtract)

        # ---- h += relu(A*agg + B)   (transpose to feature-major, fused BN)
        Wn = W_sb[layer + 1] if layer < 3 else None
        for t in range(TPC):
            trp = ps_agg.tile([128, 128], F16, tag=f"aggp{t % WINB}",
                              name=f"trp{t % WINB}")
            nc.tensor.transpose(out=trp[:], in_=agg_big[:, ts(t)],
                                identity=ident[:])
            y = sb.tile([128, 128], F16, tag="y", name="y")
            if t % 2 == 0:
                nc.scalar.activation(y[:], trp[:], AF.Relu, scale=A, bias=B)
            else:
                # split the BN load across ACT and DVE
                nc.vector.tensor_scalar(out=y[:], in0=trp[:], scalar1=A,
                                        scalar2=B, op0=AO.mult, op1=AO.add)
                nc.vector.tensor_scalar_max(y[:], y[:], 0.0)
            nc.vector.tensor_tensor(out=h_fm[:, ts(t)], in0=y[:],
                                    in1=h_fm[:, ts(t)], op=AO.add)
        if Wn is not None:
            for t in range(TPC):
                mmp = (ps_b if t % 2 == 0 else ps_a).tile(
                    [128, 128], F32, tag="mmp", name="mmp")
                nc.tensor.matmul(out=mmp[:], lhsT=h_fm[:, ts(t)], rhs=Wn[:],
                                 start=True, stop=True)
                if t % 2 == 0:
                    nc.scalar.activation(tshard_sb[:, ts(t)], mmp[:], AF.Copy,
                                         scale=dinv_sb[:, t:t + 1])
                else:
                    nc.vector.tensor_scalar(
                        out=tshard_sb[:, ts(t)], in0=mmp[:],
                        scalar1=dinv_sb[:, t:t + 1], scalar2=None,
                        op0=AO.mult)
            flush_tshard(tables[layer + 1])

    # ------------------------------------------------------------------
    # head: out = tanh(relu(BN(h3 @ Wf1)) @ Wf2 + bf2)
    # ------------------------------------------------------------------
    fsum_ps = ps_st.tile([32, 1], F32, tag="sum", name="fsum")
    fsq_ps = ps_st.tile([32, 1], F32, tag="sq", name="fsq")
    for t in range(TPC):
        fp = (ps_b if t % 2 == 0 else ps_a).tile(
            [128, 32], F32, tag="mmp", name="fp")
        nc.tensor.matmul(out=fp[:], lhsT=h_fm[:, ts(t)], rhs=Wf1_sb[:],
                         start=True, stop=True)
        fs = slice(t * 32, (t + 1) * 32)
        nc.vector.tensor_scalar(out=fpre_big[:, fs], in0=fp[:],
                                scalar1=msk_sb[:, t:t + 1], scalar2=None,
                                op0=AO.mult)
    for t in range(TPC):
        fs = slice(t * 32, (t + 1) * 32)
        sq = sb.tile([128, 32], F16, tag="sq32", name="sq32")
        nc.vector.tensor_tensor(out=sq[:], in0=fpre_big[:, fs],
                                in1=fpre_big[:, fs], op=AO.mult)
        nc.tensor.matmul(out=fsum_ps[:], lhsT=fpre_big[:, fs], rhs=ones_col[:],
                         start=(t == 0), stop=(t == TPC - 1))
        nc.tensor.matmul(out=fsq_ps[:], lhsT=sq[:], rhs=ones_col[:],
                         start=(t == 0), stop=(t == TPC - 1))

    nc.vector.tensor_copy(st_l[4][:32, 0:1], fsum_ps[:])
    nc.vector.tensor_copy(st_l[4][:32, 1:2], fsq_ps[:])
    fgl = exchange_stats(4)

    fstat = sb.tile([32, 6], F32, tag="bn", name="fbn")
    mu, ex2, var, rs, A, B = (fstat[:, i:i + 1] for i in range(6))
    nc.vector.tensor_scalar_mul(mu, fgl[:32, 0:1], 1.0 / N_NODES)
    nc.vector.tensor_scalar_mul(ex2, fgl[:32, 1:2], 1.0 / N_NODES)
    nc.vector.tensor_tensor(out=var, in0=mu, in1=mu, op=AO.mult)
    nc.vector.tensor_tensor(out=var, in0=ex2, in1=var, op=AO.subtract)
    fsd = sb.tile([32, 1], F32, tag="sd", name="fsd")
    nc.scalar.activation(fsd[:], var, AF.Sqrt, bias=eps_col[:32, :])
    nc.vector.reciprocal(rs, fsd[:])
    nc.vector.tensor_tensor(out=A, in0=rs, in1=gf_sb[:], op=AO.mult)
    nc.vector.tensor_tensor(out=B, in0=mu, in1=A, op=AO.mult)
    nc.vector.tensor_tensor(out=B, in0=btf_sb[:], in1=B, op=AO.subtract)

    for t in range(TPC):
        fs = slice(t * 32, (t + 1) * 32)
        trf = ps_agg.tile([32, 128], F16, tag=f"aggp{t % WINB}",
                          name=f"trf{t % WINB}")
        nc.tensor.transpose(out=trf[:], in_=fpre_big[:, fs], identity=ident[:])
        f_fm = sb.tile([33, 128], F16, tag="f", name="f")
        if t % 2 == 0:
            nc.scalar.activation(f_fm[:32, :], trf[:], AF.Relu,
                                 scale=A, bias=B)
        else:
            nc.vector.tensor_scalar(out=f_fm[:32, :], in0=trf[:], scalar1=A,
                                    scalar2=B, op0=AO.mult, op1=AO.add)
            nc.vector.tensor_scalar_max(f_fm[:32, :], f_fm[:32, :], 0.0)
        nc.vector.memset(f_fm[32:33, :], 1.0)
        op = (ps_b if t % 2 == 0 else ps_a).tile(
            [128, 2], F32, tag="mmp", name="op")
        nc.tensor.matmul(out=op[:], lhsT=f_fm[:], rhs=Wf2_sb[:],
                         start=True, stop=True)
        nc.scalar.activation(out_nm[:, 2 * t:2 * t + 2], op[:], AF.Tanh)
    nc.sync.dma_start(t_out[:], out_nm[:])


# ----------------------------------------------------------------------------
# Public entry point
# ----------------------------------------------------------------------------

_CACHE = {}

_PREP_VERSION = "v7_q4"


def _get_compiled(edge_index, x):
    key = hash((edge_index.tobytes(), x.shape))
    if key not in _CACHE:
        import os
        plan = streams = None
        cpath = None
        if os.environ.get("KERNEL_DEV_CACHE"):
            import pickle
            cpath = f"/tmp/prep_{_PREP_VERSION}_{key & 0xFFFFFFFF:x}.pkl"
            if os.path.exists(cpath):
                with open(cpath, "rb") as f:
                    plan, streams = pickle.load(f)
        if plan is None:
            plan, streams = _preprocess(edge_index, x)
            if cpath:
                import pickle
                with open(cpath, "wb") as f:
                    pickle.dump((plan, streams), f)
        nc = _build_program(plan)
        _CACHE.clear()
        _CACHE[key] = (nc, streams)
    return _CACHE[key]


def _in_maps(streams, kw):
    rep = dict(
        We=np.asarray(kw["We"], np.float16),
        W1=np.asarray(kw["W1"], np.float16),
        W2=np.asarray(kw["W2"], np.float16),
        W3=np.asarray(kw["W3"], np.float16),
        Wf1=np.asarray(kw["Wf1"], np.float16),
        Wf2e=np.concatenate(
            [np.asarray(kw["Wf2"], np.float32),
             np.asarray(kw["bf2"], np.float32)[None, :]], 0
        ).astype(np.float16),
        be_col=np.asarray(kw["be"], np.float32)[:, None],
        g1=np.asarray(kw["g1"], np.float32)[:, None],
        bt1=np.asarray(kw["bt1"], np.float32)[:, None],
        g2=np.asarray(kw["g2"], np.float32)[:, None],
        bt2=np.asarray(kw["bt2"], np.float32)[:, None],
        g3=np.asarray(kw["g3"], np.float32)[:, None],
        bt3=np.asarray(kw["bt3"], np.float32)[:, None],
        gf=np.asarray(kw["gf"], np.float32)[:, None],
        btf=np.asarray(kw["btf"], np.float32)[:, None],
    )
    return [dict(rep, **streams[c]) for c in range(NCORES)]


def run(trace=False, tmpdir=None, **kw):
    x = np.asarray(kw["x"], np.float32)
    edge_index = np.asarray(kw["edge_index"], np.int32)
    nc, streams = _get_compiled(edge_index, x)
    res = run_bass_kernel_spmd(nc, _in_maps(streams, kw),
                               core_ids=list(range(NCORES)), trace=trace,
                               tmpdir=tmpdir)
    shards = []
    for c in range(NCORES):
        buf = res.results[c]["out"]                   # [128, TPC*2]
        nm = buf.reshape(128, TPC, 2).transpose(1, 0, 2).reshape(NPAD, 2)
        shards.append(nm[:NSH])
    out = np.ascontiguousarray(np.concatenate(shards, 0))
    return out, res


def kernel(**kw):
    out, _ = run(trace=False, **kw)
    return out



# revision 17
# speedup vs baseline: 1.7402x; 1.1965x over previous
"""Trainium2 Bass kernel for nn_EnhancedGNN (3-block GCN + BN + MLP head).

Strategy (8 NeuronCores, node-partitioned graph parallel):
  - Nodes are partitioned contiguously across the 8 cores (12500 each,
    padded to 12544 = 98*128). Each core owns all edges whose dst lies in
    its partition. Self-loops never enter the message stream: their
    contribution (dinv^2 * h@W) is added per window from the local shard.
  - Per GCN layer, each core holds a full fp16 feature table
    H~ = dinv * (h @ W) in its DRAM, replicated NOT via collectives but by
    an XOR-pair RDMA exchange: step m swaps half-shards with core (own^m)
    through SBUF slots (remote_dma_broadcast), paced by per-round ack
    semaphores; cross-die dests carry a ^2 routing compensation. Each
    core's table stores shard s at region (s ^ own), row-interleaved so
    drains write contiguous 25KB runs. Messages are fetched with
    dma_gather (random 256B rows, int16 group-relative indices, trailing
    pads negative + per-core valid counts in a register stream).
  - Segment-sum by dst is a one-hot matmul: for each chunk of 128
    dst-sorted messages, S[msg, dstslot] = (dstrel[msg] == iota[slot])
    built on DVE, then PSUM += S^T @ msgs on TensorE. dinv[dst] is applied
    on PSUM evacuation; dinv[src] is folded into the table rows; the GCN
    bias is absorbed by the following BatchNorm.
  - h lives FEATURE-major ([feat, node]) so the h @ W table matmuls need
    no transpose, and BN+ReLU fuses into one per-partition scale/bias op
    (alternating between ACT and DVE to balance engine load). BN stats
    are column vectors from ones-matmuls, summed across cores by tiny
    RDMA broadcasts with per-phase semaphores.
"""

import numpy as np

import concourse.bacc as bacc
import concourse.mybir as mybir
import concourse.tile as tile
from concourse.bass import AP
from concourse.bass_utils import run_bass_kernel_spmd
from concourse.masks import make_identity

F32 = mybir.dt.float32
F16 = mybir.dt.float16
I16 = mybir.dt.int16
I32 = mybir.dt.int32
I8 = mybir.dt.int8

N_NODES = 100000
HID = 128
NCORES = 8
EPS = 1e-5
RG = [list(range(NCORES))]
WINB = 4                         # windows per PSUM batch ([128, 512] tile)
KMAX = 32                        # chunks per dma_gather call


def _set_config(n_nodes=100000, group=32768):
    """Derive all sharding constants (module globals) from the node count."""
    global N_NODES, NSH, TPC, NPAD, TROWS, GROUP, GBASES, GSIZES, NG
    N_NODES = n_nodes
    NSH = N_NODES // NCORES
    TPC = (NSH + 127) // 128
    NPAD = TPC * 128
    TROWS = NCORES * NPAD
    GROUP = group
    GBASES = list(range(0, TROWS, GROUP))
    GSIZES = [min(GROUP, TROWS - b) for b in GBASES]
    NG = len(GBASES)


_set_config()


# ----------------------------------------------------------------------------
# Host-side preprocessing: edge bucketing + per-core streams
# ----------------------------------------------------------------------------

def _preprocess(edge_index, x):
    # Self-loops are excluded from the gather stream: their source rows are
    # the core's own tshard_sb (already in SBUF) and are added on-device as
    # dinv * tshard per window. Degrees still count the loops.
    src = edge_index[0].astype(np.int64)
    dst = edge_index[1].astype(np.int64)

    deg = (np.bincount(dst, minlength=N_NODES) + 1).astype(np.float32)
    dinv = (1.0 / np.sqrt(deg)).astype(np.float32)

    owner = src // NSH
    loc = src % NSH
    # XOR-permuted region + row-interleaved layout: on core c, the shard of
    # core s lives at table rows [(s ^ c) * NPAD, ...), row = (loc%128)*TPC
    # + loc//128 so the RDMA drain writes contiguous 25KB runs per partition.
    rowin = (loc % 128) * TPC + (loc // 128)
    core = dst // NSH
    dloc = dst % NSH
    win = dloc // 128
    drel = dloc % 128

    counts = np.zeros((NCORES, TPC, NG), np.int64)
    percore = []
    for c in range(NCORES):
        m = core == c
        s_c = (owner[m] ^ c) * NPAD + rowin[m]   # per-core table row id
        g_c = s_c // GROUP
        w_c, d_c = win[m], drel[m]
        o = np.lexsort((d_c, g_c, w_c))
        s_c, w_c, g_c, d_c = s_c[o], w_c[o], g_c[o], d_c[o]
        cnt = np.zeros((TPC, NG), np.int64)
        np.add.at(cnt, (w_c, g_c), 1)
        counts[c] = cnt
        percore.append((s_c, d_c, cnt))
    cmax = counts.max(0)
    chunks = -(-cmax // 128)                     # [TPC, NG] ceil
    assert (chunks.sum(1) > 0).all()

    # ---- static call plan shared by every core
    win_total = chunks.sum(1)
    batches = []
    bucket_gids = {}                             # (w, g) -> [chunk gids]
    gid = 0
    ncalls = 0
    nb = -(-TPC // WINB)
    for b in range(nb):
        wins = list(range(b * WINB, min((b + 1) * WINB, TPC)))
        emitted = {w: 0 for w in wins}
        calls = []
        chunk0 = gid
        for g in range(NG):
            pend = []
            for w in wins:
                if chunks[w, g]:
                    bucket_gids.setdefault((w, g), [])
                    pend.extend([w] * chunks[w, g])
            for i0 in range(0, len(pend), KMAX):
                sub = pend[i0:i0 + KMAX]
                ch = []
                for j, w in enumerate(sub):
                    st = emitted[w] == 0
                    emitted[w] += 1
                    # stop is carried by the self-loop identity matmul that
                    # closes each window's accumulation in the evac section
                    ch.append((w, st, False))
                    bucket_gids[(w, g)].append(gid + j)
                calls.append(dict(g=g, k=len(sub), chunks=ch, gid0=gid,
                                  cid=ncalls))
                ncalls += 1
                gid += len(sub)
        batches.append(dict(wins=wins, calls=calls, chunk0=chunk0,
                            nchunks=gid - chunk0))
    tot_chunks = gid

    # ---- per-core streams
    streams = []
    for c in range(NCORES):
        s_c, d_c, cnt = percore[c]
        # offsets of each (w, g) bucket in the lexsorted arrays
        flat = cnt.reshape(-1)
        offs = np.concatenate([[0], np.cumsum(flat)[:-1]]).reshape(TPC, NG)
        # pad slots carry idx -1: the gather ucode skips negative indices,
        # saving the padding fetch bandwidth (S is zero there regardless)
        idx_stream = np.full((tot_chunks, 128), -1, np.int16)
        drel_stream = np.full((tot_chunks, 128), -1.0, np.float32)
        for (w, g), gids in bucket_gids.items():
            n = cnt[w, g]
            ncap = len(gids) * 128
            o0 = offs[w, g]
            vals = np.full(ncap, -1, np.int64)
            vals[:n] = s_c[o0:o0 + n] - GBASES[g]
            dr = np.full(ncap, -1.0, np.float32)
            dr[:n] = d_c[o0:o0 + n]
            idx_stream[gids] = vals.reshape(-1, 128).astype(np.int16)
            drel_stream[gids] = dr.reshape(-1, 128)
        # Reorder each call's chunks so bucket-tail (likely-partial) chunks
        # land at the call's end where the trailing-negative skip (ncnt)
        # can cut them. Shared across cores (same static plan); applied to
        # core 0's pass only via reorder_done guard below.
        if c == 0:
            for b in batches:
                for call in b["calls"]:
                    k, g0, g = call["k"], call["gid0"], call["g"]
                    rank = []
                    for j, (w, _st, sp) in enumerate(call["chunks"]):
                        gl = bucket_gids[(w, g)]
                        rank.append(len(gl) - 1 - gl.index(g0 + j))
                    perm = sorted(range(k), key=lambda j: (-rank[j], j))
                    call["perm"] = perm
            # start flag goes to each window's first chunk in execution order
            for b in batches:
                seen = set()
                for call in b["calls"]:
                    ch = [call["chunks"][j] for j in call["perm"]]
                    newch = []
                    for (w, _st, sp) in ch:
                        newch.append((w, w not in seen, sp))
                        seen.add(w)
                    call["chunks"] = newch
        for b in batches:
            for call in b["calls"]:
                k, g0 = call["k"], call["gid0"]
                perm = call["perm"]
                idx_stream[g0:g0 + k] = idx_stream[[g0 + j for j in perm]]
                drel_stream[g0:g0 + k] = drel_stream[[g0 + j for j in perm]]
        # Negative idxs are skipped by the gather ucode, but only as a
        # trailing suffix: keep -1 for each call's tail pads, remap interior
        # pads to dummy row 0, and pass count = last valid + 1 per call.
        ncnt = np.zeros((1, ncalls), np.int32)
        for b in batches:
            for call in b["calls"]:
                g0, k = call["gid0"], call["k"]
                blk = idx_stream[g0:g0 + k].reshape(-1)
                nz = np.nonzero(blk >= 0)[0]
                L = int(nz[-1]) + 1 if len(nz) else 0
                head = blk[:L]
                head[head < 0] = 0
                blk[:L] = head
                idx_stream[g0:g0 + k] = blk.reshape(k, 128)
                ncnt[0, call["cid"]] = L
        # wrap indices: chunk j, msg p -> [p % 16, j*8 + p//16]; the x8
        # partition replication the gather ucode wants is done on-device
        idx16 = np.ascontiguousarray(
            idx_stream.reshape(tot_chunks * 8, 16).T)             # [16, 8*T]
        drel_cols = np.ascontiguousarray(drel_stream.T).astype(np.int8)
        dv = np.zeros(NPAD, np.float32)
        dv[:NSH] = dinv[c * NSH:(c + 1) * NSH]
        dinv_cols = np.ascontiguousarray(dv.reshape(TPC, 128).T)  # [128, TPC]
        mk = np.zeros(NPAD, np.float32)
        mk[:NSH] = 1.0
        msk_cols = np.ascontiguousarray(mk.reshape(TPC, 128).T)
        xp = np.zeros((NPAD, 2), np.float32)
        xp[:NSH] = x[c * NSH:(c + 1) * NSH]
        xT = np.ascontiguousarray(xp.T).astype(np.float16)        # [2, NPAD]
        streams.append(dict(idxs=idx16, drel=drel_cols,
                            dinv=dinv_cols, msk=msk_cols, xT=xT, ncnt=ncnt))

    plan = dict(batches=batches, tot_chunks=tot_chunks,
                tot_cols=tot_chunks * 8, ncalls=ncalls,
                maxnch=max(b["nchunks"] for b in batches))
    return plan, streams


# ----------------------------------------------------------------------------
# Device program
# ----------------------------------------------------------------------------

NQ = 4                           # SWDGE queues: gathers on 0..NQ-2, RDMA on NQ-1


def _blob_layout(plan):
    """All per-core inputs packed into ONE int8 DRAM tensor: per-tensor h2d
    transfers carry a large fixed cost through the axon tunnel, and 21
    tensors x 8 cores of upload skew was most of the kernel's wall time."""
    spec = [
        ("idxs", (16, plan["tot_cols"]), np.int16, I16),
        ("ncnt", (1, plan["ncalls"]), np.int32, I32),
        ("drel", (128, plan["tot_chunks"]), np.int8, I8),
        ("dinv", (128, TPC), np.float32, F32),
        ("msk", (128, TPC), np.float32, F32),
        ("xT", (2, NPAD), np.float16, F16),
        ("We", (2, HID), np.float16, F16),
        ("W1", (HID, HID), np.float16, F16),
        ("W2", (HID, HID), np.float16, F16),
        ("W3", (HID, HID), np.float16, F16),
        ("Wf1", (HID, 32), np.float16, F16),
        ("Wf2e", (33, 2), np.float16, F16),
        ("be_col", (HID, 1), np.float32, F32),
        ("g1", (HID, 1), np.float32, F32),
        ("bt1", (HID, 1), np.float32, F32),
        ("g2", (HID, 1), np.float32, F32),
        ("bt2", (HID, 1), np.float32, F32),
        ("g3", (HID, 1), np.float32, F32),
        ("bt3", (HID, 1), np.float32, F32),
        ("gf", (32, 1), np.float32, F32),
        ("btf", (32, 1), np.float32, F32),
    ]
    layout = {}
    off = 0
    for name, shape, npdt, dt in spec:
        nbytes = int(np.prod(shape)) * np.dtype(npdt).itemsize
        layout[name] = (off, shape, npdt, dt)
        off += (nbytes + 255) // 256 * 256
    return layout, off


def _build_program(plan):
    nc = bacc.Bacc("TRN2", target_bir_lowering=False, debug=False,
                   enable_asserts=True, num_devices=NCORES,
                   num_swdge_queues=NQ)

    layout, blob_bytes = _blob_layout(plan)
    t_blob = nc.dram_tensor("blob", [1, blob_bytes], I8,
                            kind="ExternalInput").ap()

    def din(name):
        off, shape, npdt, dt = layout[name]
        nbytes = int(np.prod(shape)) * np.dtype(npdt).itemsize
        v = t_blob[0:1, off:off + nbytes].bitcast(dt)
        return v.rearrange("o (p f) -> (o p) f", p=shape[0])

    t_idx = din("idxs")
    t_ncnt = din("ncnt")
    t_drel = din("drel")
    t_dinv = din("dinv")
    t_msk = din("msk")
    t_xT = din("xT")
    t_We = din("We")
    t_W = {1: din("W1"), 2: din("W2"), 3: din("W3")}
    t_Wf1 = din("Wf1")
    t_Wf2 = din("Wf2e")
    t_be = din("be_col")
    t_g = {i: din(f"g{i}") for i in (1, 2, 3)}
    t_bt = {i: din(f"bt{i}") for i in (1, 2, 3)}
    t_gf = din("gf")
    t_btf = din("btf")
    t_out = nc.dram_tensor("out", [128, TPC * 2], F32,
                           kind="ExternalOutput").ap()

    from contextlib import ExitStack
    with tile.TileContext(nc) as tc, ExitStack() as st:
        cst = st.enter_context(tc.tile_pool(name="cst", bufs=1))
        sb = st.enter_context(tc.tile_pool(name="sb", bufs=2))
        msgp = st.enter_context(tc.tile_pool(name="msgp", bufs=4))
        ps_agg = st.enter_context(tc.tile_pool(name="ps_agg", bufs=1, space="PSUM"))
        ps_st = st.enter_context(tc.tile_pool(name="ps_st", bufs=1, space="PSUM"))
        ps_a = st.enter_context(tc.tile_pool(name="ps_a", bufs=1, space="PSUM"))
        ps_b = st.enter_context(tc.tile_pool(name="ps_b", bufs=1, space="PSUM"))
        dr = st.enter_context(tc.tile_pool(name="dr", bufs=1, space="DRAM"))
        _emit(nc, tc, plan, locals())
    nc.compile()
    return nc


def _emit(nc, tc, plan, pools):
    cst, sb, msgp = pools["cst"], pools["sb"], pools["msgp"]
    ps_agg, ps_st = pools["ps_agg"], pools["ps_st"]
    ps_a, ps_b, dr = pools["ps_a"], pools["ps_b"], pools["dr"]
    t_idx, t_drel = pools["t_idx"], pools["t_drel"]
    t_ncnt = pools["t_ncnt"]
    t_dinv, t_xT, t_We = pools["t_dinv"], pools["t_xT"], pools["t_We"]
    t_W, t_Wf1, t_Wf2 = pools["t_W"], pools["t_Wf1"], pools["t_Wf2"]
    t_be, t_g, t_bt = pools["t_be"], pools["t_g"], pools["t_bt"]
    t_gf, t_btf = pools["t_gf"], pools["t_btf"]
    t_out = pools["t_out"]
    AO, AF = mybir.AluOpType, mybir.ActivationFunctionType

    # ---- constants
    iota_i = cst.tile([128, 128], I32)
    nc.gpsimd.iota(iota_i[:], pattern=[[1, 128]], base=0, channel_multiplier=0)
    iota_f32 = cst.tile([128, 128], F32)
    nc.vector.tensor_copy(iota_f32[:], iota_i[:])
    iota_f = cst.tile([128, 128], F16)
    nc.vector.tensor_copy(iota_f[:], iota_f32[:])
    ident = cst.tile([128, 128], F16)
    make_identity(nc, ident[:])
    ones_col = cst.tile([128, 1], F16)
    nc.vector.memset(ones_col[:], 1.0)
    eps_col = cst.tile([128, 1], F32)
    nc.vector.memset(eps_col[:], EPS)

    _lc = [0]

    def load_const(t, shape, dt=F32):
        _lc[0] += 1
        tl = cst.tile(shape, dt, name=f"c{_lc[0]}")
        nc.sync.dma_start(tl[:], t[:])
        return tl

    dinv_sb = load_const(t_dinv, [128, TPC])
    ncnt_sb = load_const(t_ncnt, [1, plan["ncalls"]], I32)
    ncnt_reg = nc.gpsimd.alloc_register("ncnt_reg")

    # idx stream: uploaded once as [16, 8T]; the x8 partition replication
    # the gather ucode expects is materialized by 8 strided DMAs here.
    idxb_all = cst.tile([128, plan["tot_cols"]], I16)
    for r in range(8):
        nc.sync.dma_start(idxb_all[16 * r:16 * (r + 1), :], t_idx[:])
    drel_i8 = cst.tile([128, plan["tot_chunks"]], I8)
    nc.sync.dma_start(drel_i8[:], t_drel[:])
    drel_all = cst.tile([128, plan["tot_chunks"]], F16)
    nc.vector.tensor_copy(drel_all[:], drel_i8[:])
    msk_sb = load_const(pools["t_msk"], [128, TPC])
    We_sb = load_const(t_We, [2, HID], F16)
    W_sb = {i: load_const(t_W[i], [HID, HID], F16) for i in (1, 2, 3)}
    Wf1_sb = load_const(t_Wf1, [HID, 32], F16)
    Wf2_sb = load_const(t_Wf2, [33, 2], F16)
    be_sb = load_const(t_be, [HID, 1])
    g_sb = {i: load_const(t_g[i], [HID, 1]) for i in (1, 2, 3)}
    bt_sb = {i: load_const(t_bt[i], [HID, 1]) for i in (1, 2, 3)}
    gf_sb = load_const(t_gf, [32, 1])
    btf_sb = load_const(t_btf, [32, 1])

    h_fm = cst.tile([128, NPAD], F16)        # FEATURE-major h [feat, node]
    agg_big = cst.tile([128, NPAD], F16)     # node-major aggregation output
    fpre_big = cst.tile([128, TPC * 32], F16)
    tshard_sb = cst.tile([128, NPAD], F16)   # staging for the table shard
    out_nm = cst.tile([128, TPC * 2], F32)   # node-major staged output
    HTPC = TPC // 2
    slot_t = [cst.tile([128, HTPC * 128], F16, name=f"slot{p}")
              for p in (0, 1)]
    st_l = {i: cst.tile([128, 2], F32, name=f"st{i}") for i in (1, 2, 3, 4)}
    slot_st = {i: cst.tile([128, 14], F32, name=f"sst{i}")
               for i in (1, 2, 3, 4)}

    tables = {i: dr.tile([TROWS, HID], F16, name=f"table{i}")
              for i in (1, 2, 3)}

    import os
    SIM_STUB = bool(os.environ.get("KERNEL_SIM_STUB"))

    # XOR-pair exchange: step m swaps shards with core (own ^ m). Cross-die
    # dests (bit 2) get an extra ^2 from the ucode lane balance; compensate
    # (the interpreter models ideal routing — KERNEL_RDMA_NOCOMP disables it).
    NOCOMP = bool(os.environ.get("KERNEL_RDMA_NOCOMP"))

    def rd(m):
        if NOCOMP:
            return (0, m)
        return (0, m ^ 2 if m >= 4 else m)

    RD1 = {m: [None] * 8 for m in range(1, 8)}
    for m in range(1, 8):
        RD1[m][m] = rd(m)
    RD_ALL = [None] + [rd(k) for k in range(1, 8)]

    nc.vector.memset(st_l[4][:], 0.0)
    for _ in range(4):
        # initialize all three rotating gather buffers: rows of slots the
        # gather skips (trailing pads) are read by the matmul (times S=0)
        mz = msgp.tile([128, KMAX * 128], F16, tag="msg", name="msg")
        nc.vector.memset(mz[:], 0.0)

    rsem_t = [nc.alloc_semaphore("rsem_t0"), nc.alloc_semaphore("rsem_t1")]
    rsem_s = {i: nc.alloc_semaphore(f"rsem_s{i}") for i in (1, 2, 3, 4)}
    lsem_t = nc.alloc_semaphore("lsem_t")
    lsem_s = nc.alloc_semaphore("lsem_s")
    ack_sems = [nc.alloc_semaphore(f"ack{j}") for j in range(14)]
    ack_lsem = nc.alloc_semaphore("ack_lsem")
    psem = nc.alloc_semaphore("psem")
    tsem = nc.alloc_semaphore("tsem")
    cnt = dict(prep=0, send_t=0, arr0=0, arr1=0, drain=0, flush=0)

    def prep_trig(inst):
        inst.then_inc(psem, 1)
        cnt["prep"] += 1
        nc.gpsimd.wait_ge(psem, cnt["prep"])
        nc.gpsimd.trigger_dma(count=1, queue_num=NQ - 1)

    def ts(t):
        return slice(t * 128, (t + 1) * 128)

    def region(table, m):
        return table[m * NPAD:(m + 1) * NPAD, :].rearrange(
            "(p w) f -> p w f", w=TPC)

    def flush_tshard(table):
        """XOR-pair RDMA exchange of tshard_sb into the local table copy.

        Half-shard rounds: round r = (m-1)*2 + h sends window-half h of the
        shard to peer (own ^ m), 2-slot pipelined with a global drain-ack
        barrier gating slot reuse (round r waits acks of rounds <= r-2)."""
        if SIM_STUB:
            # Cost-sim stand-in: the local drains only (the broadcasts and
            # handshake can't be modeled by a single-core timeline sim).
            for m in range(8):
                nc.sync.dma_start(
                    region(table, m),
                    tshard_sb[:].rearrange("p (w f) -> p w f", f=HID))
            return
        F = cnt["flush"]
        cnt["flush"] += 1
        with tc.tile_critical():
            nc.sync.dma_start(
                region(table, 0),
                tshard_sb[:].rearrange("p (w f) -> p w f", f=HID)
            ).then_inc(tsem, 16)
            cnt["drain"] += 16
            for m in range(1, 8):
                for h in (0, 1):
                    j = (m - 1) * 2 + h          # round within this flush
                    # slot-reuse barrier: round j-2 (same flush; or j+12 of
                    # the previous flush) drained on every core
                    if not SIM_STUB and (F > 0 or j >= 2):
                        if j >= 2:
                            nc.gpsimd.wait_ge(ack_sems[j - 2], 14 * (F + 1))
                        else:
                            nc.gpsimd.wait_ge(ack_sems[j + 12], 14 * F)
                    cs = slice(h * HTPC * 128, (h + 1) * HTPC * 128)
                    prep_trig(nc.gpsimd.remote_dma_broadcast(
                        out_ap=slot_t[j % 2][:], in_ap=tshard_sb[:, cs],
                        remote_sem=rsem_t[j % 2], local_sem=lsem_t,
                        rdests=RD1[m], queue_num=NQ - 1))
                    cnt["send_t"] += 16
                    if j > 0:
                        # ack round j-1 (its drain was issued last iteration)
                        nc.gpsimd.wait_ge(tsem, cnt["drain"])
                        prep_trig(nc.gpsimd.remote_sem_update_broadcast(
                            ack_sems[j - 1], ack_lsem, rdests=RD_ALL,
                            queue_num=NQ - 1))
                    par = j % 2
                    key = "arr0" if par == 0 else "arr1"
                    cnt[key] += 2
                    if not SIM_STUB:
                        nc.sync.wait_ge(rsem_t[par], cnt[key])
                    nc.sync.dma_start(
                        region(table, m)[:, h * HTPC:(h + 1) * HTPC, :],
                        slot_t[par][:].rearrange("p (w f) -> p w f", f=HID)
                    ).then_inc(tsem, 16)
                    cnt["drain"] += 16
            nc.gpsimd.wait_ge(tsem, cnt["drain"])
            prep_trig(nc.gpsimd.remote_sem_update_broadcast(
                ack_sems[13], ack_lsem, rdests=RD_ALL, queue_num=NQ - 1))
            nc.sync.wait_ge(tsem, cnt["drain"])
            nc.gpsimd.wait_ge(lsem_t, cnt["send_t"])

    def exchange_stats(phase):
        """Sum [128, 2] column stats across cores via RDMA broadcasts."""
        gl = sb.tile([128, 2], F32, tag="stv", name=f"glv{phase}")
        if SIM_STUB:
            nc.vector.tensor_copy(gl[:], st_l[phase][:])
            return gl
        with tc.tile_critical():
            for m in range(1, 8):
                prep_trig(nc.gpsimd.remote_dma_broadcast(
                    out_ap=slot_st[phase][:, 2 * (m - 1):2 * m],
                    in_ap=st_l[phase][:], remote_sem=rsem_s[phase],
                    local_sem=lsem_s, rdests=RD1[m], queue_num=NQ - 1))
            if not SIM_STUB:
                nc.vector.wait_ge(rsem_s[phase], 14)
        nc.vector.tensor_tensor(out=gl[:], in0=st_l[phase][:],
                                in1=slot_st[phase][:, 0:2], op=AO.add)
        for m in range(2, 8):
            nc.vector.tensor_tensor(
                out=gl[:], in0=gl[:],
                in1=slot_st[phase][:, 2 * (m - 1):2 * m], op=AO.add)
        return gl

    # ------------------------------------------------------------------
    # embed: h = relu(x @ We + be)   (feature-major)
    # ------------------------------------------------------------------
    for t in range(TPC):
        xt_t = sb.tile([2, 128], F16, tag="xt", name="xt")
        nc.sync.dma_start(xt_t[:], t_xT[:, ts(t)])
        mmp = (ps_b if t % 2 == 0 else ps_a).tile(
            [128, 128], F32, tag="mmp", name="mmp_e")
        nc.tensor.matmul(out=mmp[:], lhsT=We_sb[:], rhs=xt_t[:],
                         start=True, stop=True)
        if t % 2 == 0:
            nc.scalar.activation(h_fm[:, ts(t)], mmp[:], AF.Relu,
                                 bias=be_sb[:])
        else:
            nc.vector.tensor_scalar(out=h_fm[:, ts(t)], in0=mmp[:],
                                    scalar1=be_sb[:], scalar2=None,
                                    op0=AO.add)
            nc.vector.tensor_scalar_max(h_fm[:, ts(t)], h_fm[:, ts(t)], 0.0)
    for t in range(TPC):
        mm2 = (ps_a if t % 2 == 0 else ps_b).tile(
            [128, 128], F32, tag="mmp", name="mmp2_e")
        nc.tensor.matmul(out=mm2[:], lhsT=h_fm[:, ts(t)], rhs=W_sb[1][:],
                         start=True, stop=True)
        if t % 2 == 0:
            nc.scalar.activation(tshard_sb[:, ts(t)], mm2[:], AF.Copy,
                                 scale=dinv_sb[:, t:t + 1])
        else:
            nc.vector.tensor_scalar(out=tshard_sb[:, ts(t)], in0=mm2[:],
                                    scalar1=dinv_sb[:, t:t + 1], scalar2=None,
                                    op0=AO.mult)
    flush_tshard(tables[1])

    # ------------------------------------------------------------------
    # 3 GCN blocks
    # ------------------------------------------------------------------
    for layer in (1, 2, 3):
        sum_ps = ps_st.tile([128, 1], F32, tag="sum", name=f"sum{layer}")
        sq_ps = ps_st.tile([128, 1], F32, tag="sq", name=f"sq{layer}")
        for batch in plan["batches"]:
            aggp = {wl: ps_agg.tile([128, 128], F32, tag=f"aggp{wl}",
                                    name=f"aggp{wl}")
                    for wl in range(len(batch["wins"]))}
            for call in batch["calls"]:
                k, g, gid0 = call["k"], call["g"], call["gid0"]
                msg = msgp.tile([128, k * 128], F16, tag="msg", name="msg",
                                padded_shape=[128, KMAX * 128])
                cid = call["cid"]
                nc.gpsimd.reg_load(ncnt_reg, ncnt_sb[0:1, cid:cid + 1])
                nc.gpsimd.dma_gather(
                    out_ap=msg[:].rearrange("p (c e) -> p c e", e=HID),
                    in_ap=tables[layer][GBASES[g]:GBASES[g] + GSIZES[g], :],
                    idxs_ap=idxb_all[:, gid0 * 8:(gid0 + k) * 8],
                    num_idxs=k * 128, num_idxs_reg=ncnt_reg, elem_size=HID,
                    single_packet=False, queue_num=cid % (NQ - 1))
                scall = sb.tile([128, k * 128], F16, tag="scall", name="scall",
                                padded_shape=[128, KMAX * 128])
                drs = drel_all[:, gid0:gid0 + k]
                in0 = AP(iota_f[:].tensor, iota_f[:].offset,
                         [iota_f[:].ap[0], [0, k], iota_f[:].ap[1]])
                in1 = AP(drs.tensor, drs.offset,
                         [drs.ap[0], drs.ap[1], [0, 128]])
                nc.vector.tensor_tensor(
                    out=scall[:].rearrange("p (c e) -> p c e", e=128),
                    in0=in0, in1=in1, op=AO.is_equal)
                for j, (w, st, sp) in enumerate(call["chunks"]):
                    wl = w % WINB
                    nc.tensor.matmul(
                        out=aggp[wl][:],
                        lhsT=scall[:, j * 128:(j + 1) * 128],
                        rhs=msg[:, j * 128:(j + 1) * 128],
                        start=st, stop=sp)
            for w in batch["wins"]:
                wl = w % WINB
                # self-loop term: ident @ tshard adds dinv*(h@W) into the
                # window's PSUM; the evac's dinv scale makes it dinv^2*(h@W).
                # This matmul also closes the accumulation group (stop).
                nc.tensor.matmul(out=aggp[wl][:], lhsT=ident[:],
                                 rhs=tshard_sb[:, ts(w)],
                                 start=False, stop=True)
                nc.scalar.activation(agg_big[:, ts(w)], aggp[wl][:],
                                     AF.Copy, scale=dinv_sb[:, w:w + 1])
                sq = sb.tile([128, 128], F16, tag="sq", name="sqt")
                nc.vector.tensor_tensor(out=sq[:], in0=agg_big[:, ts(w)],
                                        in1=agg_big[:, ts(w)], op=AO.mult)
                nc.tensor.matmul(out=sum_ps[:], lhsT=agg_big[:, ts(w)],
                                 rhs=ones_col[:],
                                 start=(w == 0), stop=(w == TPC - 1))
                nc.tensor.matmul(out=sq_ps[:], lhsT=sq[:], rhs=ones_col[:],
                                 start=(w == 0), stop=(w == TPC - 1))

        # ---- BN stats exchange ([128, 2] column stats)
        nc.vector.tensor_copy(st_l[layer][:, 0:1], sum_ps[:])
        nc.vector.tensor_copy(st_l[layer][:, 1:2], sq_ps[:])
        gl_sb = exchange_stats(layer)

        # ---- BN affine coefficients A, B [128, 1]
        stat = sb.tile([128, 6], F32, tag="bn", name="bn")
        mu, ex2, var, rs, A, B = (stat[:, i:i + 1] for i in range(6))
        nc.vector.tensor_scalar_mul(mu, gl_sb[:, 0:1], 1.0 / N_NODES)
        nc.vector.tensor_scalar_mul(ex2, gl_sb[:, 1:2], 1.0 / N_NODES)
        nc.vector.tensor_tensor(out=var, in0=mu, in1=mu, op=AO.mult)
        nc.vector.tensor_tensor(out=var, in0=ex2, in1=var, op=AO.subtract)
        sd = sb.tile([128, 1], F32, tag="sd", name="sd")
        nc.scalar.activation(sd[:], var, AF.Sqrt, bias=eps_col[:])
        nc.vector.reciprocal(rs, sd[:])
        nc.vector.tensor_tensor(out=A, in0=rs, in1=g_sb[layer][:], op=AO.mult)
        nc.vector.tensor_tensor(out=B, in0=mu, in1=A, op=AO.mult)
        nc.vector.tensor_tensor(out=B, in0=bt_sb[layer][:], in1=B,
                                op=AO.subtract)

        # ---- h += relu(A*agg + B)   (transpose to feature-major, fused BN)
        Wn = W_sb[layer + 1] if layer < 3 else None
        for t in range(TPC):
            trp = ps_agg.tile([128, 128], F16, tag=f"aggp{t % WINB}",
                              name=f"trp{t % WINB}")
            nc.tensor.transpose(out=trp[:], in_=agg_big[:, ts(t)],
                                identity=ident[:])
            y = sb.tile([128, 128], F16, tag="y", name="y")
            if t % 2 == 0:
                nc.scalar.activation(y[:], trp[:], AF.Relu, scale=A, bias=B)
            else:
                # split the BN load across ACT and DVE
                nc.vector.tensor_scalar(out=y[:], in0=trp[:], scalar1=A,
                                        scalar2=B, op0=AO.mult, op1=AO.add)
                nc.vector.tensor_scalar_max(y[:], y[:], 0.0)
            nc.vector.tensor_tensor(out=h_fm[:, ts(t)], in0=y[:],
                                    in1=h_fm[:, ts(t)], op=AO.add)
        if Wn is not None:
            for t in range(TPC):
                mmp = (ps_b if t % 2 == 0 else ps_a).tile(
                    [128, 128], F32, tag="mmp", name="mmp")
                nc.tensor.matmul(out=mmp[:], lhsT=h_fm[:, ts(t)], rhs=Wn[:],
                                 start=True, stop=True)
                if t % 2 == 0:
                    nc.scalar.activation(tshard_sb[:, ts(t)], mmp[:], AF.Copy,
                                         scale=dinv_sb[:, t:t + 1])
                else:
                    nc.vector.tensor_scalar(
                        out=tshard_sb[:, ts(t)], in0=mmp[:],
                        scalar1=dinv_sb[:, t:t + 1], scalar2=None,
                        op0=AO.mult)
            flush_tshard(tables[layer + 1])

    # ------------------------------------------------------------------
    # head: out = tanh(relu(BN(h3 @ Wf1)) @ Wf2 + bf2)
    # ------------------------------------------------------------------
    fsum_ps = ps_st.tile([32, 1], F32, tag="sum", name="fsum")
    fsq_ps = ps_st.tile([32, 1], F32, tag="sq", name="fsq")
    for t in range(TPC):
        fp = (ps_b if t % 2 == 0 else ps_a).tile(
            [128, 32], F32, tag="mmp", name="fp")
        nc.tensor.matmul(out=fp[:], lhsT=h_fm[:, ts(t)], rhs=Wf1_sb[:],
                         start=True, stop=True)
        fs = slice(t * 32, (t + 1) * 32)
        nc.vector.tensor_scalar(out=fpre_big[:, fs], in0=fp[:],
                                scalar1=msk_sb[:, t:t + 1], scalar2=None,
                                op0=AO.mult)
    for t in range(TPC):
        fs = slice(t * 32, (t + 1) * 32)
        sq = sb.tile([128, 32], F16, tag="sq32", name="sq32")
        nc.vector.tensor_tensor(out=sq[:], in0=fpre_big[:, fs],
                                in1=fpre_big[:, fs], op=AO.mult)
        nc.tensor.matmul(out=fsum_ps[:], lhsT=fpre_big[:, fs], rhs=ones_col[:],
                         start=(t == 0), stop=(t == TPC - 1))
        nc.tensor.matmul(out=fsq_ps[:], lhsT=sq[:], rhs=ones_col[:],
                         start=(t == 0), stop=(t == TPC - 1))

    nc.vector.tensor_copy(st_l[4][:32, 0:1], fsum_ps[:])
    nc.vector.tensor_copy(st_l[4][:32, 1:2], fsq_ps[:])
    fgl = exchange_stats(4)

    fstat = sb.tile([32, 6], F32, tag="bn", name="fbn")
    mu, ex2, var, rs, A, B = (fstat[:, i:i + 1] for i in range(6))
    nc.vector.tensor_scalar_mul(mu, fgl[:32, 0:1], 1.0 / N_NODES)
    nc.vector.tensor_scalar_mul(ex2, fgl[:32, 1:2], 1.0 / N_NODES)
    nc.vector.tensor_tensor(out=var, in0=mu, in1=mu, op=AO.mult)
    nc.vector.tensor_tensor(out=var, in0=ex2, in1=var, op=AO.subtract)
    fsd = sb.tile([32, 1], F32, tag="sd", name="fsd")
    nc.scalar.activation(fsd[:], var, AF.Sqrt, bias=eps_col[:32, :])
    nc.vector.reciprocal(rs, fsd[:])
    nc.vector.tensor_tensor(out=A, in0=rs, in1=gf_sb[:], op=AO.mult)
    nc.vector.tensor_tensor(out=B, in0=mu, in1=A, op=AO.mult)
    nc.vector.tensor_tensor(out=B, in0=btf_sb[:], in1=B, op=AO.subtract)

    for t in range(TPC):
        fs = slice(t * 32, (t + 1) * 32)
        trf = ps_agg.tile([32, 128], F16, tag=f"aggp{t % WINB}",
                          name=f"trf{t % WINB}")
        nc.tensor.transpose(out=trf[:], in_=fpre_big[:, fs], identity=ident[:])
        f_fm = sb.tile([33, 128], F16, tag="f", name="f")
        if t % 2 == 0:
            nc.scalar.activation(f_fm[:32, :], trf[:], AF.Relu,
                                 scale=A, bias=B)
        else:
            nc.vector.tensor_scalar(out=f_fm[:32, :], in0=trf[:], scalar1=A,
                                    scalar2=B, op0=AO.mult, op1=AO.add)
            nc.vector.tensor_scalar_max(f_fm[:32, :], f_fm[:32, :], 0.0)
        nc.vector.memset(f_fm[32:33, :], 1.0)
        op = (ps_b if t % 2 == 0 else ps_a).tile(
            [128, 2], F32, tag="mmp", name="op")
        nc.tensor.matmul(out=op[:], lhsT=f_fm[:], rhs=Wf2_sb[:],
                         start=True, stop=True)
        nc.scalar.activation(out_nm[:, 2 * t:2 * t + 2], op[:], AF.Tanh)
    nc.sync.dma_start(t_out[:], out_nm[:])


# ----------------------------------------------------------------------------
# Public entry point
# ----------------------------------------------------------------------------

_CACHE = {}

_PREP_VERSION = "v7_q4"


def _get_compiled(edge_index, x):
    key = hash((edge_index.tobytes(), x.shape))
    if key not in _CACHE:
        import os
        plan = streams = None
        cpath = None
        if os.environ.get("KERNEL_DEV_CACHE"):
            import pickle
            cpath = f"/tmp/prep_{_PREP_VERSION}_{key & 0xFFFFFFFF:x}.pkl"
            if os.path.exists(cpath):
                with open(cpath, "rb") as f:
                    plan, streams = pickle.load(f)
        if plan is None:
            plan, streams = _preprocess(edge_index, x)
            if cpath:
                import pickle
                with open(cpath, "wb") as f:
                    pickle.dump((plan, streams), f)
        nc = _build_program(plan)
        _CACHE.clear()
        _CACHE[key] = (nc, streams, plan)
    return _CACHE[key]


def _in_maps(plan, streams, kw):
    rep = dict(
        We=np.asarray(kw["We"], np.float16),
        W1=np.asarray(kw["W1"], np.float16),
        W2=np.asarray(kw["W2"], np.float16),
        W3=np.asarray(kw["W3"], np.float16),
        Wf1=np.asarray(kw["Wf1"], np.float16),
        Wf2e=np.concatenate(
            [np.asarray(kw["Wf2"], np.float32),
             np.asarray(kw["bf2"], np.float32)[None, :]], 0
        ).astype(np.float16),
        be_col=np.asarray(kw["be"], np.float32)[:, None],
        g1=np.asarray(kw["g1"], np.float32)[:, None],
        bt1=np.asarray(kw["bt1"], np.float32)[:, None],
        g2=np.asarray(kw["g2"], np.float32)[:, None],
        bt2=np.asarray(kw["bt2"], np.float32)[:, None],
        g3=np.asarray(kw["g3"], np.float32)[:, None],
        bt3=np.asarray(kw["bt3"], np.float32)[:, None],
        gf=np.asarray(kw["gf"], np.float32)[:, None],
        btf=np.asarray(kw["btf"], np.float32)[:, None],
    )
    layout, blob_bytes = _blob_layout(plan)
    maps = []
    for c in range(NCORES):
        vals = dict(rep, **streams[c])
        blob = np.zeros((1, blob_bytes), np.int8)
        for name, (off, shape, npdt, _dt) in layout.items():
            a = np.ascontiguousarray(np.asarray(vals[name], npdt))
            assert a.shape == shape, (name, a.shape, shape)
            raw = a.view(np.int8).reshape(-1)
            blob[0, off:off + raw.size] = raw
        maps.append(dict(blob=blob))
    return maps


def run(trace=False, tmpdir=None, **kw):
    x = np.asarray(kw["x"], np.float32)
    edge_index = np.asarray(kw["edge_index"], np.int32)
    nc, streams, plan = _get_compiled(edge_index, x)
    res = run_bass_kernel_spmd(nc, _in_maps(plan, streams, kw),
                               core_ids=list(range(NCORES)), trace=trace,
                               tmpdir=tmpdir)
    shards = []
    for c in range(NCORES):
        buf = res.results[c]["out"]                   # [128, TPC*2]
        nm = buf.reshape(128, TPC, 2).transpose(1, 0, 2).reshape(NPAD, 2)
        shards.append(nm[:NSH])
    out = np.ascontiguousarray(np.concatenate(shards, 0))
    return out, res


def kernel(**kw):
    out, _ = run(trace=False, **kw)
    return out

